# revision 33
# baseline (speedup 1.0000x reference)
"""Trainium2 Bass kernel for nn_AttentionBlock (B=4, C=512, N=2048, H=8, DK=64).

Computation (see module docstring of the reference):
  xt = x.T; qkv = xt @ Wp.T + bp; per head: S[j,i] = k_j . q_i / 8,
  P = softmax over i (query axis => per-j rows of S^T), O = P^T-weighted
  v-mix, out = (O @ Wo.T + bo + xt).T.

Sharding: 8 cores = (batch b = core//2) x (head-group g = core%2, 4 heads).
Each core emits two f16 partial resT [C, N] tensors (one per head pair);
host sums partials (x8 scale), adds bias + residual.

Engine strategy (cost-model driven):
  - All heavy matmuls are fp8e4 DoubleRow (0.5 cyc/row): QK/V projections
    (host supplies x/W pre-packed [64, 2, .] k-tiles), S (q/k re-packed to
    [32, 2, N] via SBUF->SBUF DMA), PV (contracts jt PAIRS: K=256 as
    [128, 2, .]), out-projection (o folded to [64, 2, N] via DMA).
  - exp work is split across THREE engines per (jtpair, head) unit:
      ACT: native Exp (scale=1/8, bias=-2ln2 so E<=61 fits e4m3),
           fp8 E tiles + free accum_out row sums -> DR PV.
      DVE/Pool: Schraudolph bit-trick exp: i16 = floor(A*S + B16) is the
           bit pattern of f16(~exp(S/8)); sums via a DVE tensor_scalar
           accum pass (4x mode); PV in fp16 for these tiles.
  - Normalization folds into v: vp = v * (VP/sumE); PV accumulates
    VP-scaled O in PSUM; o-evac rescales by 8/VP into fp8; host /8.
  - PSUM: 2 rotating [128,1024] S-slots (ACT + DVE consumers) + two
    per-head [64,1024] O-half accumulators (DR matmuls cannot write at a
    partition offset); PV runs in two i-half passes.
"""

import math
import os
import numpy as np
import ml_dtypes

import concourse.bass as bass
import concourse.tile as tile
from concourse import bacc, mybir
from concourse.bass_utils import run_bass_kernel_spmd

F32 = mybir.dt.float32
F16 = mybir.dt.float16
F8 = mybir.dt.float8e4
I16 = mybir.dt.int16
AF = mybir.ActivationFunctionType
ALU = mybir.AluOpType
DR = mybir.MatmulPerfMode.DoubleRow

B, C, N = 4, 512, 2048
H, DK = 8, 64
N_CORES = 8
SCALE = DK ** -0.5              # 0.125
EXPB = -2.0 * math.log(2.0)     # ACT exp bias: E' = exp(z)/4 (max ~61 < 240)
A_SCH = 1024.0 * 1.4426950408889634 * SCALE   # Schraudolph slope on raw S
B_SCH = 15294.0                 # tuned offset (incl trunc+centering)
VP = 512.0                      # vp pre-scale (fp8 precision for v/sumE)
OEV = 8.0 / VP                  # o-evac rescale: o8 = 8*O_true; host /8
HOST_DIV = 8.0

# per (pair, jtpair t, head h): engine for the exp unit.
# 'A' = ACT native exp (fp8 E, DR PV); 'D' = DVE Schraudolph (f16 E, fp16
# PV). GPSIMD cannot read PSUM (BIR verifier), so Pool only gets the
# SBUF-side work: sum passes over f16 E tiles, vp scaling, ssum adds.
ASSIGN = [
    [('A','A'),('A','D'),('D','A'),('A','D'),('D','A'),('A','D'),('D','A'),('A','A')],
    [('A','A'),('A','D'),('D','A'),('A','D'),('D','A'),('A','D'),('D','A'),('A','A')],
]

LAST_RESULT = None
_NC = None


def _build_nc():
    nc = bacc.Bacc("TRN2", target_bir_lowering=False, debug=False,
                   num_devices=N_CORES)

    x8 = nc.dram_tensor("x8", [64, 4, 2, N], F8, kind="ExternalInput").ap()
    wqk8 = nc.dram_tensor("wqk8", [64, 4, 2, 4, 128], F8, kind="ExternalInput").ap()
    bqk = nc.dram_tensor("bqk", [128, 4], F32, kind="ExternalInput").ap()
    wv8 = nc.dram_tensor("wv8", [64, 4, 2, 256], F8, kind="ExternalInput").ap()
    bpv = nc.dram_tensor("bpv", [1, 256], F16, kind="ExternalInput").ap()
    wo8 = nc.dram_tensor("wo8", [64, 2, 2, C], F8, kind="ExternalInput").ap()
    ones = nc.dram_tensor("ones", [1, 128], F16, kind="ExternalInput").ap()
    out_a = nc.dram_tensor("out_a", [C, N], F16, kind="ExternalOutput").ap()
    out_b = nc.dram_tensor("out_b", [C, N], F16, kind="ExternalOutput").ap()

    with tile.TileContext(nc) as tc:
        with (
            tc.tile_pool(name="consts", bufs=1) as consts,
            tc.tile_pool(name="qkpool", bufs=1) as qkpool,
            tc.tile_pool(name="vpool", bufs=1) as vpool,
            tc.tile_pool(name="e8pool", bufs=11) as e8pool,
            tc.tile_pool(name="e16pool", bufs=7) as e16pool,
            tc.tile_pool(name="vppool", bufs=18) as vppool,
            tc.tile_pool(name="opool", bufs=2) as opool,
            tc.tile_pool(name="outpool", bufs=2) as outpool,
            tc.tile_pool(name="smalls", bufs=40) as smalls,
            tc.tile_pool(name="psum", bufs=1, space="PSUM") as pp,
        ):
            ones_sb = consts.tile([1, 128], F16)
            nc.sync.dma_start(ones_sb[:], ones[:])
            bqk_sb = consts.tile([128, 4], F32)
            nc.sync.dma_start(bqk_sb[:], bqk[:])
            bpv_sb = consts.tile([1, 256], F16)
            nc.sync.dma_start(bpv_sb[:], bpv[:])
            wqk_sb = consts.tile([64, 4, 2, 4, 128], F8)
            nc.sync.dma_start(wqk_sb[:], wqk8[:])
            x_sb = consts.tile([64, 4, 2, N], F8)
            for kt in range(4):
                nc.sync.dma_start(x_sb[:, kt], x8[:, kt])
            wv_sb = consts.tile([64, 4, 2, 256], F8)
            nc.sync.dma_start(wv_sb[:], wv8[:])
            wo_sb = consts.tile([64, 2, 2, C], F8)
            nc.sync.dma_start(wo_sb[:], wo8[:])

            # warm the ACT exp table while DMAs run
            warm = smalls.tile([1, 128], F16, tag="warm", bufs=1, name="warm")
            nc.scalar.activation(warm[:], ones_sb[:], AF.Exp)
            expb_sb = consts.tile([128, 1], F32)
            nc.vector.memset(expb_sb[:], EXPB)
            zrow_sb = consts.tile([1, 512], F16)
            nc.vector.memset(zrow_sb[:], 0.0)

            # persistent SBUF tensors
            # qk8: fp8 evac of the QK projection [128 feat, N], rotated
            qk8_of = {}

            def qk8_tile(ft):
                if ft not in qk8_of:
                    qk8_of[ft] = qkpool.tile([128, N], F8, tag="qk8e",
                                             bufs=2, name="qk8e")
                return qk8_of[ft]
            # qh8: S-DR layout [32, (head,qk) 8, kk 2, N]
            qh8 = qkpool.tile([32, 8, 2, N], F8, name="qh8")
            v16 = vpool.tile([128, 16, 256], F16, name="v16")
            # o8f: per-pair, per-head fp8 o evac [64 d, N] (partition 0-63)
            o8f_of = {}

            def o8f_tile(p, h):
                if (p, h) not in o8f_of:
                    o8f_of[(p, h)] = opool.tile([64, N], F8, tag=f"o8f{h}",
                                                bufs=1, name="o8f")
                return o8f_of[(p, h)]
            # o8d: DR-folded [64, 2, N]
            o8d_of = {}

            def o8d_tile(p):
                if p not in o8d_of:
                    o8d_of[p] = opool.tile([64, 2, N], F8, tag="o8d", bufs=1,
                                           name="o8d")
                return o8d_of[p]
            scr16 = qkpool.tile([128, 2048], F16, name="scr16")

            def s_slot():
                return pp.tile([128, 1024], F32, tag="s", bufs=2, name="s_ps")

            def qk_proj(ft, ih):
                # one i-half of ftile ft -> PSUM -> fp8 evac into qk8[:, ft]
                ps = s_slot()
                for ic in range(4):
                    nch = ih * 1024 + ic * 256
                    for kt in range(4):
                        nc.tensor.matmul(
                            ps[:, ic * 256:(ic + 1) * 256],
                            lhsT=wqk_sb[:, kt, :, ft, :],
                            rhs=x_sb[:, kt, :, nch:nch + 256],
                            start=(kt == 0), stop=(kt == 3), perf_mode=DR,
                        )
                nc.vector.tensor_scalar(
                    qk8_tile(ft)[:, ih * 1024:(ih + 1) * 1024],
                    ps[:], bqk_sb[:, ft:ft + 1], None, ALU.add)

            def qk_reshape(pair, which):
                # fold [128, N] ftile into S-DR layout [32, 2, N] per head
                ft = 2 * pair + which
                src = qk8_tile(ft)
                for lh in range(2):
                    ht = (2 * pair + lh) * 2 + which
                    for kk in range(2):
                        base = lh * 64 + kk * 32
                        nc.sync.dma_start(
                            qh8[:, ht, kk, :], src[base:base + 32, :])

            def v_proj(nt):
                ps = s_slot()
                for kt in range(4):
                    nc.tensor.matmul(
                        ps[:, :256],
                        lhsT=x_sb[:, kt, :, nt * 128:(nt + 1) * 128],
                        rhs=wv_sb[:, kt], start=(kt == 0), stop=False,
                        perf_mode=DR,
                    )
                nc.tensor.matmul(
                    ps[:, :256], lhsT=ones_sb[:1, :], rhs=bpv_sb[:1, :],
                    start=False, stop=True,
                )
                if nt % 2 == 0:
                    nc.scalar.copy(v16[:, nt, :], ps[:, :256])
                else:
                    nc.vector.tensor_copy(v16[:, nt, :], ps[:, :256])

            op_count = [0]

            def out_proj(pair, cot, ih):
                # [128 c, 1024 i] DR out-proj chunk + f16 evac + DMA
                dst = out_a if pair == 0 else out_b
                ps = s_slot()
                for ic in range(4):
                    nc.tensor.matmul(
                        ps[:, ic * 256:(ic + 1) * 256],
                        lhsT=wo_sb[:, pair, :, cot * 128:(cot + 1) * 128],
                        rhs=o8d_tile(pair)[:, :, ih * 1024 + ic * 256:
                                      ih * 1024 + (ic + 1) * 256],
                        start=True, stop=True, perf_mode=DR,
                    )
                ot = outpool.tile([128, 1024], F16, tag="out", name="out_t")
                eng = [nc.scalar.copy,
                       nc.vector.tensor_copy][op_count[0] % 2]
                op_count[0] += 1
                eng(ot[:], ps[:])
                nc.sync.dma_start(
                    dst[cot * 128:(cot + 1) * 128,
                        ih * 1024:(ih + 1) * 1024], ot[:])

            class Unit:
                """One (pair, jtpair, head) exp unit."""

                def __init__(self, pair, t, h):
                    self.pair, self.t, self.h = pair, t, h
                    self.eng = ASSIGN[pair][t][h]
                    self.ht_q = (2 * pair + h) * 2
                    self.ht_k = self.ht_q + 1
                    if self.eng == 'A':
                        self.e8 = e8pool.tile([128, 2, N], F8, tag="e8",
                                              name="e8")
                    else:
                        self.e16 = e16pool.tile([128, 2, N], I16, tag="e16",
                                                name="e16")
                    self.accs = {}
                    self.recs = {}
                    self.vps = None

                def s_and_consume(self):
                    # 4 PSUM tiles: (kk=jt-of-pair, ih=i-half)
                    for kk in range(2):
                        for ih in range(2):
                            self.s_tile(kk, ih)

                def s_tile(self, kk, ih):
                    # one [128 j, 1024 i] S fill + its exp/affine consumer
                    jt = 2 * self.t + kk
                    if True:
                        if True:
                            ps = s_slot()
                            for ic in range(4):
                                i0 = ih * 1024 + ic * 256
                                nc.tensor.matmul(
                                    ps[:, ic * 256:(ic + 1) * 256],
                                    lhsT=qh8[:, self.ht_k, :,
                                             jt * 128:(jt + 1) * 128],
                                    rhs=qh8[:, self.ht_q, :, i0:i0 + 256],
                                    start=True, stop=True, perf_mode=DR,
                                )
                            if self.eng == 'A':
                                acc = smalls.tile([128, 1], F32, tag="acc",
                                                  bufs=12, name="acc")
                                nc.scalar.activation(
                                    self.e8[:, kk, ih * 1024:(ih + 1) * 1024],
                                    ps[:], AF.Exp, scale=SCALE,
                                    bias=expb_sb[:], accum_out=acc,
                                )
                                self.accs[(kk, ih)] = acc
                            else:
                                nc.vector.tensor_scalar(
                                    self.e16[:, kk, ih * 1024:(ih + 1) * 1024],
                                    ps[:], A_SCH, B_SCH, ALU.mult, ALU.add)

                def sums_and_vp(self):
                    vps = vppool.tile([128, 2, 64], F8 if self.eng == 'A'
                                      else F16, tag="vp", name="vp")
                    self.vps = vps
                    for kk in range(2):
                        jt = 2 * self.t + kk
                        ssum = smalls.tile([128, 1], F32, tag="ssum", bufs=12,
                                           name="ssum")
                        if self.eng == 'A':
                            nc.gpsimd.tensor_add(
                                ssum[:], self.accs[(kk, 0)][:],
                                self.accs[(kk, 1)][:])
                        else:
                            nc.vector.tensor_scalar(
                                scr16[:], self.e16[:, kk, :].bitcast(F16),
                                1.0, None, ALU.mult, ALU.add,
                                accum_out=ssum[:])
                        rec = smalls.tile([128, 1], F32, tag="rec", bufs=12,
                                          name="rec")
                        nc.vector.reciprocal(rec[:], ssum[:])
                        lv = 2 * self.pair + self.h
                        nc.gpsimd.tensor_scalar(
                            vps[:, kk, :],
                            v16[:, jt, lv * 64:(lv + 1) * 64],
                            rec[:], VP, ALU.mult, ALU.mult,
                        )

                def pv(self, ih, ohs):
                    stop = (self.t == 7)
                    o_ps = ohs[self.h]
                    if self.eng == 'A':
                        for ic in range(4):
                            i0 = ih * 1024 + ic * 256
                            nc.tensor.matmul(
                                o_ps[:, ic * 256:(ic + 1) * 256],
                                lhsT=self.vps[:],
                                rhs=self.e8[:, :, i0:i0 + 256],
                                start=False, stop=stop, perf_mode=DR,
                                skip_group_check=True,
                            )
                    else:
                        ef = self.e16[:].bitcast(F16)
                        for kk in range(2):
                            for q in range(2):
                                i0 = ih * 1024 + q * 512
                                nc.tensor.matmul(
                                    o_ps[:, q * 512:(q + 1) * 512],
                                    lhsT=self.vps[:, kk, :],
                                    rhs=ef[:, kk, i0:i0 + 512],
                                    start=False, stop=(stop and kk == 1),
                                    skip_group_check=True,
                                )

            def oh_alloc():
                # fresh per-head O-half accumulators (both on partitions
                # 0-63: DR matmuls cannot target a partition offset).
                # zero-matmuls set has_written so every PV is an accumulate.
                ohs = []
                for h in range(2):
                    o_ps = pp.tile([64, 1024], F32, tag=f"oh{h}", bufs=1,
                                   name=f"o_ps{h}")
                    for q in range(2):
                        nc.tensor.matmul(
                            o_ps[:, q * 512:(q + 1) * 512],
                            lhsT=zrow_sb[:1, :64], rhs=zrow_sb[:1, :],
                            start=True, stop=False, skip_group_check=True,
                        )
                    ohs.append(o_ps)
                return ohs

            def o_evac(pair, ih, ohs):
                for h in range(2):
                    nc.vector.tensor_scalar(
                        o8f_tile(pair, h)[:, ih * 1024:(ih + 1) * 1024],
                        ohs[h][:], OEV, None, ALU.mult)

            def o_fold(pair):
                for kk in range(2):
                    nc.sync.dma_start(
                        o8d_tile(pair)[:, kk, :], o8f_tile(pair, kk)[:, :])

            # ---------------- emission ----------------
            # prologue: pair0 q/k projections + reshape
            for ft in (0, 1):
                for ih in range(2):
                    qk_proj(ft, ih)
            qk_reshape(0, 0)
            qk_reshape(0, 1)
            v_proj(0)
            v_proj(1)
            v_proj(2)
            v_proj(3)

            # aux work queue: emitted interleaved under pair0 attention.
            # v_proj(jt) must be emitted before sums of unit t=jt//2.
            aux = [lambda n=nt: v_proj(n) for nt in range(4, 16)]
            for ft in (2, 3):
                for ih in range(2):
                    aux.append(lambda f=ft, i=ih: qk_proj(f, i))
            aux.append(lambda: qk_reshape(1, 0))
            aux.append(lambda: qk_reshape(1, 1))

            def pop_aux(k):
                for _ in range(k):
                    if aux:
                        aux.pop(0)()

            units = {}
            order = [(t, h) for t in range(8) for h in (0, 1)]
            TILES = [(0, 0), (0, 1), (1, 0), (1, 1)]

            def mk_groups(pair):
                us = [Unit(pair, t, h) for (t, h) in order]
                for u in us:
                    units[(pair, u.t, u.h)] = u
                aq = [u for u in us if u.eng == 'A']
                dq = [u for u in us if u.eng == 'D']
                gs = []
                # 2 ACT units per DVE unit keeps both engines saturated
                # (the DVE unit also carries its accum pass + evac aux)
                pat = [(2, 1), (1, 1), (2, 1), (1, 1), (2, 1), (2, 1)]
                for na, nd in pat:
                    g = [aq.pop(0) for _ in range(min(na, len(aq)))]
                    g += [dq.pop(0) for _ in range(min(nd, len(dq)))]
                    if g:
                        gs.append(g)
                while aq or dq:
                    g = []
                    if aq:
                        g.append(aq.pop(0))
                    if dq:
                        g.append(dq.pop(0))
                    gs.append(g)
                return gs

            # ---- pair 0: tile-interleaved groups, proj/v aux between ----
            oh0 = oh_alloc()
            pend = None
            for g in mk_groups(0):
                for kk, ih in TILES:
                    for u in g:
                        u.s_tile(kk, ih)
                if pend:
                    for u in pend:
                        u.pv(0, oh0)
                for u in g:
                    u.sums_and_vp()
                pend = g
                pop_aux(3)
            for u in pend:
                u.pv(0, oh0)
            while aux:
                pop_aux(1)
            o_evac(0, 0, oh0)

            # ---- pair 1, interleaved with pair0 phase C + out-proj ----
            oh1 = oh_alloc()
            q_p0c = [lambda tt=tt, hh=hh: units[(0, tt, hh)].pv(1, oh1)
                     for tt in range(8) for hh in (0, 1)]
            state = {"oh_p1": None}

            def oh_p1():
                if state["oh_p1"] is None:
                    state["oh_p1"] = oh_alloc()
                return state["oh_p1"]

            q_p1a = []      # pair1 pv-half0, appended as units complete
            q_op0 = []      # pair0 out-proj, filled after o_fold(0)

            def drain(last=False):
                n = 0
                while q_p0c and n < 4:
                    q_p0c.pop(0)()
                    n += 1
                if not q_p0c and "p0_done" not in state:
                    o_evac(0, 1, oh1)
                    o_fold(0)
                    q_op0.extend(lambda c=cot, i=ih2: out_proj(0, c, i)
                                 for cot in range(4) for ih2 in range(2))
                    state["p0_done"] = True
                    return
                if "p0_done" in state:
                    while q_p1a and n < 5:
                        q_p1a.pop(0)()
                        n += 1
                    if (not q_p1a or last) and q_op0:
                        q_op0.pop(0)()

            for g in mk_groups(1):
                for kk, ih in TILES:
                    for u in g:
                        u.s_tile(kk, ih)
                for u in g:
                    u.sums_and_vp()
                    q_p1a.append(lambda p=u: p.pv(0, oh_p1()))
                drain()
            while q_p0c or q_p1a:
                drain(last=True)
            o_evac(1, 0, state["oh_p1"])

            # ---- pair 1 phase C + tail ----
            oh3 = oh_alloc()
            for t in range(8):
                for h in (0, 1):
                    units[(1, t, h)].pv(1, oh3)
                    if q_op0:
                        q_op0.pop(0)()
            o_evac(1, 1, oh3)
            o_fold(1)
            while q_op0:
                q_op0.pop(0)()
            for cot in range(4):
                for ih in range(2):
                    out_proj(1, cot, ih)

    nc.compile()
    return nc


def get_nc():
    global _NC
    if _NC is None:
        _NC = _build_nc()
    return _NC


def core_inputs(x, Wp, bp, core):
    """Host-side shard prep for one core: b = core//2, g = core%2."""
    b, g = divmod(core, 2)
    E4 = ml_dtypes.float8_e4m3

    def to8(a):
        return np.ascontiguousarray(np.asarray(a, np.float32).astype(E4))

    xb = x[b]  # [C, N]
    # x8[p, kt, kk, n] = x[kt*128 + kk*64 + p, n]
    x8 = np.transpose(xb.reshape(4, 2, 64, N), (2, 0, 1, 3))

    # qk feature order: ftile ft = 2*pair + (0=q,1=k); within: hA d0-63, hB
    qidx = np.zeros((4, 128), np.int64)
    for pair in range(2):
        for which in range(2):
            ft = 2 * pair + which
            for lh in range(2):
                h = 4 * g + 2 * pair + lh
                base = h * 192 + which * 64
                qidx[ft, lh * 64:(lh + 1) * 64] = np.arange(base, base + 64)
    Wqk = Wp[qidx.reshape(-1)]            # [512 feat, C]
    # wqk8[p, kt, kk, ft, j] = Wqk[ft*128 + j, kt*128 + kk*64 + p]
    wqk8 = np.transpose(
        Wqk.reshape(4, 128, 4, 2, 64), (4, 2, 3, 0, 1))
    bqk = bp[qidx.reshape(-1)].reshape(4, 128).T  # [128, 4]

    vidx = np.concatenate([np.arange((4 * g + lh) * 192 + 128,
                                     (4 * g + lh) * 192 + 192)
                           for lh in range(4)])
    Wv = Wp[vidx]                          # [256, C]
    # wv8[p, kt, kk, f] = Wv[f, kt*128 + kk*64 + p]
    wv8 = np.transpose(Wv.reshape(256, 4, 2, 64), (3, 1, 2, 0))

    return {
        "x8": to8(x8),
        "wqk8": to8(wqk8),
        "bqk": np.ascontiguousarray(bqk.astype(np.float32)),
        "wv8": to8(wv8),
        "bpv": bp[vidx].astype(np.float16).reshape(1, 256),
        "ones": np.ones((1, 128), np.float16),
    }


def wo_inputs(Wo, core):
    g = core % 2
    E4 = ml_dtypes.float8_e4m3
    # wo8[p, pair, kk, c] = Wo[c, 256*g + pair*128 + kk*64 + p]
    Wog = Wo[:, 256 * g:256 * (g + 1)]     # [C, 256]
    wo8 = np.transpose(Wog.reshape(C, 2, 2, 64), (3, 1, 2, 0))
    return np.ascontiguousarray(np.asarray(wo8, np.float32).astype(E4))


def kernel(x, Wp, bp, Wo, bo):
    global LAST_RESULT
    x = np.asarray(x, dtype=np.float32)
    Wp = np.asarray(Wp, dtype=np.float32)
    bp = np.asarray(bp, dtype=np.float32)
    Wo = np.asarray(Wo, dtype=np.float32)
    bo = np.asarray(bo, dtype=np.float32)

    in_maps = []
    for core in range(N_CORES):
        m = core_inputs(x, Wp, bp, core)
        m["wo8"] = wo_inputs(Wo, core)
        in_maps.append(m)

    nc = get_nc()
    res = run_bass_kernel_spmd(
        nc, in_maps, core_ids=list(range(N_CORES)),
        trace=bool(int(os.environ.get("KERNEL_TRACE", "0"))),
    )
    LAST_RESULT = res
    result = np.empty((B, C, N), dtype=np.float32)
    for b in range(B):
        r0, r1 = res.results[2 * b], res.results[2 * b + 1]
        result[b] = (
            (r0["out_a"].astype(np.float32) + r0["out_b"].astype(np.float32)
             + r1["out_a"].astype(np.float32) + r1["out_b"].astype(np.float32))
            / HOST_DIV
            + x[b] + bo[:, None]
        )
    return result


# revision 37
# speedup vs baseline: 1.1499x; 1.1499x over previous
"""Trainium2 Bass kernel for nn_AttentionBlock (B=4, C=512, N=2048, H=8, DK=64).

Computation (see module docstring of the reference):
  xt = x.T; qkv = xt @ Wp.T + bp; per head: S[j,i] = k_j . q_i / 8,
  P = softmax over i (query axis => per-j rows of S^T), O = P^T-weighted
  v-mix, out = (O @ Wo.T + bo + xt).T.

Sharding: 8 cores = (batch b = core//2) x (head-group g = core%2, 4 heads).
Each core emits two f16 partial resT [C, N] tensors (one per head pair);
host sums partials (x8 scale), adds bias + residual.

Engine strategy (cost-model driven):
  - All heavy matmuls are fp8e4 DoubleRow (0.5 cyc/row): QK/V projections
    (host supplies x/W pre-packed [64, 2, .] k-tiles), S (q/k re-packed to
    [32, 2, N] via SBUF->SBUF DMA), PV (contracts jt PAIRS: K=256 as
    [128, 2, .]), out-projection (o folded to [64, 2, N] via DMA).
  - exp work is split across THREE engines per (jtpair, head) unit:
      ACT: native Exp (scale=1/8, bias=-2ln2 so E<=61 fits e4m3),
           fp8 E tiles + free accum_out row sums -> DR PV.
      DVE/Pool: Schraudolph bit-trick exp: i16 = floor(A*S + B16) is the
           bit pattern of f16(~exp(S/8)); sums via a DVE tensor_scalar
           accum pass (4x mode); PV in fp16 for these tiles.
  - Normalization folds into v: vp = v * (VP/sumE); PV accumulates
    VP-scaled O in PSUM; o-evac rescales by 8/VP into fp8; host /8.
  - PSUM: 2 rotating [128,1024] S-slots (ACT + DVE consumers) + two
    per-head [64,1024] O-half accumulators (DR matmuls cannot write at a
    partition offset); PV runs in two i-half passes.
"""

import math
import os
import numpy as np
import ml_dtypes

import concourse.bass as bass
import concourse.tile as tile
from concourse import bacc, mybir
from concourse.bass_utils import run_bass_kernel_spmd

F32 = mybir.dt.float32
F16 = mybir.dt.float16
F8 = mybir.dt.float8e4
I16 = mybir.dt.int16
AF = mybir.ActivationFunctionType
ALU = mybir.AluOpType
DR = mybir.MatmulPerfMode.DoubleRow

B, C, N = 4, 512, 2048
H, DK = 8, 64
N_CORES = 8
SCALE = DK ** -0.5              # 0.125
EXPB = -2.0 * math.log(2.0)     # ACT exp bias: E' = exp(z)/4 (max ~61 < 240)
A_SCH = 1024.0 * 1.4426950408889634 * SCALE   # Schraudolph slope on raw S
B_SCH = 15294.0                 # tuned offset (incl trunc+centering)
VP = 512.0                      # vp pre-scale (fp8 precision for v/sumE)
OEV = 8.0 / VP                  # o-evac rescale: o8 = 8*O_true; host /8
HOST_DIV = 8.0

# per (pair, jtpair t, head h): engine for the exp unit.
# 'A' = ACT native exp (fp8 E, DR PV); 'D' = DVE Schraudolph (f16 E, fp16
# PV). GPSIMD cannot read PSUM (BIR verifier), so Pool only gets the
# SBUF-side work: sum passes over f16 E tiles, vp scaling, ssum adds.
ASSIGN = [
    [('A','A'),('A','D'),('D','A'),('A','D'),('D','A'),('A','D'),('D','A'),('A','A')],
    [('A','A'),('A','D'),('D','A'),('A','D'),('D','A'),('A','D'),('D','A'),('A','A')],
]

LAST_RESULT = None
_NC = None


def _build_nc():
    nc = bacc.Bacc("TRN2", target_bir_lowering=False, debug=False,
                   num_devices=N_CORES)

    x8 = nc.dram_tensor("x8", [64, 4, 2, N], F8, kind="ExternalInput").ap()
    wqk8 = nc.dram_tensor("wqk8", [64, 4, 2, 4, 128], F8, kind="ExternalInput").ap()
    bqk = nc.dram_tensor("bqk", [128, 4], F32, kind="ExternalInput").ap()
    wv8 = nc.dram_tensor("wv8", [64, 4, 2, 256], F8, kind="ExternalInput").ap()
    bpv = nc.dram_tensor("bpv", [1, 256], F16, kind="ExternalInput").ap()
    wo8 = nc.dram_tensor("wo8", [64, 2, 2, C], F8, kind="ExternalInput").ap()
    ones = nc.dram_tensor("ones", [1, 128], F16, kind="ExternalInput").ap()
    out_a = nc.dram_tensor("out_a", [C, N], F16, kind="ExternalOutput").ap()
    out_b = nc.dram_tensor("out_b", [C, N], F16, kind="ExternalOutput").ap()

    with tile.TileContext(nc) as tc:
        with (
            tc.tile_pool(name="consts", bufs=1) as consts,
            tc.tile_pool(name="qkpool", bufs=1) as qkpool,
            tc.tile_pool(name="vpool", bufs=1) as vpool,
            tc.tile_pool(name="e8pool", bufs=11) as e8pool,
            tc.tile_pool(name="e16pool", bufs=7) as e16pool,
            tc.tile_pool(name="vppool", bufs=18) as vppool,
            tc.tile_pool(name="opool", bufs=2) as opool,
            tc.tile_pool(name="outpool", bufs=2) as outpool,
            tc.tile_pool(name="smalls", bufs=40) as smalls,
            tc.tile_pool(name="psum", bufs=1, space="PSUM") as pp,
        ):
            ones_sb = consts.tile([1, 128], F16)
            nc.sync.dma_start(ones_sb[:], ones[:])
            bqk_sb = consts.tile([128, 4], F32)
            nc.sync.dma_start(bqk_sb[:], bqk[:])
            bpv_sb = consts.tile([1, 256], F16)
            nc.sync.dma_start(bpv_sb[:], bpv[:])
            wqk_sb = consts.tile([64, 4, 2, 4, 128], F8)
            nc.sync.dma_start(wqk_sb[:], wqk8[:])
            x_sb = consts.tile([64, 4, 2, N], F8)
            for kt in range(4):
                nc.sync.dma_start(x_sb[:, kt], x8[:, kt])
            wv_sb = consts.tile([64, 4, 2, 256], F8)
            nc.sync.dma_start(wv_sb[:], wv8[:])
            wo_sb = consts.tile([64, 2, 2, C], F8)
            nc.sync.dma_start(wo_sb[:], wo8[:])

            # warm the ACT exp table while DMAs run
            warm = smalls.tile([1, 128], F16, tag="warm", bufs=1, name="warm")
            nc.scalar.activation(warm[:], ones_sb[:], AF.Exp)
            expb_sb = consts.tile([128, 1], F32)
            nc.vector.memset(expb_sb[:], EXPB)
            zrow_sb = consts.tile([1, 512], F16)
            nc.vector.memset(zrow_sb[:], 0.0)

            # persistent SBUF tensors
            # qk8: fp8 evac of the QK projection [128 feat, N], rotated
            qk8_of = {}

            def qk8_tile(ft):
                if ft not in qk8_of:
                    qk8_of[ft] = qkpool.tile([128, N], F8, tag="qk8e",
                                             bufs=2, name="qk8e")
                return qk8_of[ft]
            # qh8: S-DR layout [32, (head,qk) 8, kk 2, N]
            qh8 = qkpool.tile([32, 8, 2, N], F8, name="qh8")
            v16 = vpool.tile([128, 16, 256], F16, name="v16")
            # o8f: per-pair, per-head fp8 o evac [64 d, N] (partition 0-63)
            o8f_of = {}

            def o8f_tile(p, h):
                if (p, h) not in o8f_of:
                    o8f_of[(p, h)] = opool.tile([64, N], F8, tag=f"o8f{h}",
                                                bufs=1, name="o8f")
                return o8f_of[(p, h)]
            # o8d: DR-folded [64, 2, N]
            o8d_of = {}

            def o8d_tile(p):
                if p not in o8d_of:
                    o8d_of[p] = opool.tile([64, 2, N], F8, tag="o8d", bufs=1,
                                           name="o8d")
                return o8d_of[p]
            scr16 = qkpool.tile([128, 2048], F16, name="scr16")

            def s_slot():
                # ACT-dedicated slots (also used by proj/outproj fills)
                return pp.tile([128, 1024], F32, tag="s", bufs=2, name="s_ps")

            def sd_slot():
                # DVE-dedicated half-size slots: decouples the DVE consumer
                # chain from ACT's so neither stalls the other
                return pp.tile([128, 512], F32, tag="sd", bufs=2,
                               name="sd_ps")

            def qk_proj(ft, ih):
                # one i-half of ftile ft -> PSUM -> fp8 evac into qk8[:, ft]
                ps = s_slot()
                for ic in range(4):
                    nch = ih * 1024 + ic * 256
                    for kt in range(4):
                        nc.tensor.matmul(
                            ps[:, ic * 256:(ic + 1) * 256],
                            lhsT=wqk_sb[:, kt, :, ft, :],
                            rhs=x_sb[:, kt, :, nch:nch + 256],
                            start=(kt == 0), stop=(kt == 3), perf_mode=DR,
                        )
                nc.vector.tensor_scalar(
                    qk8_tile(ft)[:, ih * 1024:(ih + 1) * 1024],
                    ps[:], bqk_sb[:, ft:ft + 1], None, ALU.add)

            def qk_reshape(pair, which):
                # fold [128, N] ftile into S-DR layout [32, 2, N] per head
                ft = 2 * pair + which
                src = qk8_tile(ft)
                for lh in range(2):
                    ht = (2 * pair + lh) * 2 + which
                    for kk in range(2):
                        base = lh * 64 + kk * 32
                        nc.sync.dma_start(
                            qh8[:, ht, kk, :], src[base:base + 32, :])

            def v_proj(nt):
                ps = s_slot()
                for kt in range(4):
                    nc.tensor.matmul(
                        ps[:, :256],
                        lhsT=x_sb[:, kt, :, nt * 128:(nt + 1) * 128],
                        rhs=wv_sb[:, kt], start=(kt == 0), stop=False,
                        perf_mode=DR,
                    )
                nc.tensor.matmul(
                    ps[:, :256], lhsT=ones_sb[:1, :], rhs=bpv_sb[:1, :],
                    start=False, stop=True,
                )
                if nt % 2 == 0:
                    nc.scalar.copy(v16[:, nt, :], ps[:, :256])
                else:
                    nc.vector.tensor_copy(v16[:, nt, :], ps[:, :256])

            op_count = [0]

            def out_proj(pair, cot, iq):
                # [128 c, 512 i] DR out-proj chunk + f16 evac + DMA
                dst = out_a if pair == 0 else out_b
                ps = s_slot()
                for ic in range(2):
                    i0 = iq * 512 + ic * 256
                    nc.tensor.matmul(
                        ps[:, ic * 256:(ic + 1) * 256],
                        lhsT=wo_sb[:, pair, :, cot * 128:(cot + 1) * 128],
                        rhs=o8d_tile(pair)[:, :, i0:i0 + 256],
                        start=True, stop=True, perf_mode=DR,
                    )
                ot = outpool.tile([128, 512], F16, tag="out", bufs=4,
                                  name="out_t")
                eng = [nc.scalar.copy,
                       nc.vector.tensor_copy][op_count[0] % 2]
                op_count[0] += 1
                eng(ot[:], ps[:, :512])
                nc.sync.dma_start(
                    dst[cot * 128:(cot + 1) * 128,
                        iq * 512:(iq + 1) * 512], ot[:])

            class Unit:
                """One (pair, jtpair, head) exp unit."""

                def __init__(self, pair, t, h):
                    self.pair, self.t, self.h = pair, t, h
                    self.eng = ASSIGN[pair][t][h]
                    self.ht_q = (2 * pair + h) * 2
                    self.ht_k = self.ht_q + 1
                    if self.eng == 'A':
                        self.e8 = e8pool.tile([128, 2, N], F8, tag="e8",
                                              name="e8")
                    else:
                        self.e16 = e16pool.tile([128, 2, N], I16, tag="e16",
                                                name="e16")
                    self.accs = {}
                    self.recs = {}
                    self.vps = None

                def s_and_consume(self):
                    # 4 PSUM tiles: (kk=jt-of-pair, ih=i-half)
                    for kk in range(2):
                        for ih in range(2):
                            self.s_tile(kk, ih)

                def s_tile(self, kk, ih):
                    # ACT: one [128 j, 1024 i] fill + exp consumer.
                    # DVE: ih indexes QUARTERS [128, 512] (8 tiles/unit).
                    jt = 2 * self.t + kk
                    if self.eng == 'A':
                        ps = s_slot()
                        for ic in range(4):
                            i0 = ih * 1024 + ic * 256
                            nc.tensor.matmul(
                                ps[:, ic * 256:(ic + 1) * 256],
                                lhsT=qh8[:, self.ht_k, :,
                                         jt * 128:(jt + 1) * 128],
                                rhs=qh8[:, self.ht_q, :, i0:i0 + 256],
                                start=True, stop=True, perf_mode=DR,
                            )
                        acc = smalls.tile([128, 1], F32, tag="acc",
                                          bufs=12, name="acc")
                        nc.scalar.activation(
                            self.e8[:, kk, ih * 1024:(ih + 1) * 1024],
                            ps[:], AF.Exp, scale=SCALE,
                            bias=expb_sb[:], accum_out=acc,
                        )
                        self.accs[(kk, ih)] = acc
                    else:
                        for q in range(2):
                            ps = sd_slot()
                            for ic in range(2):
                                i0 = ih * 1024 + q * 512 + ic * 256
                                nc.tensor.matmul(
                                    ps[:, ic * 256:(ic + 1) * 256],
                                    lhsT=qh8[:, self.ht_k, :,
                                             jt * 128:(jt + 1) * 128],
                                    rhs=qh8[:, self.ht_q, :, i0:i0 + 256],
                                    start=True, stop=True, perf_mode=DR,
                                )
                            i0 = ih * 1024 + q * 512
                            nc.vector.tensor_scalar(
                                self.e16[:, kk, i0:i0 + 512],
                                ps[:], A_SCH, B_SCH, ALU.mult, ALU.add)

                def sums_and_vp(self):
                    vps = vppool.tile([128, 2, 64], F8 if self.eng == 'A'
                                      else F16, tag="vp", name="vp")
                    self.vps = vps
                    for kk in range(2):
                        jt = 2 * self.t + kk
                        ssum = smalls.tile([128, 1], F32, tag="ssum", bufs=12,
                                           name="ssum")
                        if self.eng == 'A':
                            nc.gpsimd.tensor_add(
                                ssum[:], self.accs[(kk, 0)][:],
                                self.accs[(kk, 1)][:])
                        else:
                            nc.vector.tensor_scalar(
                                scr16[:], self.e16[:, kk, :].bitcast(F16),
                                1.0, None, ALU.mult, ALU.add,
                                accum_out=ssum[:])
                        rec = smalls.tile([128, 1], F32, tag="rec", bufs=12,
                                          name="rec")
                        nc.vector.reciprocal(rec[:], ssum[:])
                        lv = 2 * self.pair + self.h
                        nc.gpsimd.tensor_scalar(
                            vps[:, kk, :],
                            v16[:, jt, lv * 64:(lv + 1) * 64],
                            rec[:], VP, ALU.mult, ALU.mult,
                        )

                def pv(self, iq, ohs):
                    stop = (self.t == 7)
                    o_ps = ohs[self.h]
                    if self.eng == 'A':
                        for ic in range(2):
                            i0 = iq * 512 + ic * 256
                            nc.tensor.matmul(
                                o_ps[:, ic * 256:(ic + 1) * 256],
                                lhsT=self.vps[:],
                                rhs=self.e8[:, :, i0:i0 + 256],
                                start=False, stop=stop, perf_mode=DR,
                                skip_group_check=True,
                            )
                    else:
                        ef = self.e16[:].bitcast(F16)
                        i0 = iq * 512
                        for kk in range(2):
                            nc.tensor.matmul(
                                o_ps[:],
                                lhsT=self.vps[:, kk, :],
                                rhs=ef[:, kk, i0:i0 + 512],
                                start=False, stop=(stop and kk == 1),
                                skip_group_check=True,
                            )

            def oh_alloc():
                # fresh per-head O-QUARTER accumulators [64, 512] (partition
                # 0-63: DR matmuls cannot target a partition offset).
                # zero-matmuls set has_written so every PV is an accumulate.
                ohs = []
                for h in range(2):
                    o_ps = pp.tile([64, 512], F32, tag=f"oh{h}", bufs=1,
                                   name=f"o_ps{h}")
                    nc.tensor.matmul(
                        o_ps[:], lhsT=zrow_sb[:1, :64], rhs=zrow_sb[:1, :],
                        start=True, stop=False, skip_group_check=True,
                    )
                    ohs.append(o_ps)
                return ohs

            def o_evac(pair, iq, ohs):
                for h in range(2):
                    nc.vector.tensor_scalar(
                        o8f_tile(pair, h)[:, iq * 512:(iq + 1) * 512],
                        ohs[h][:], OEV, None, ALU.mult)

            def o_fold(pair, iq):
                sl = slice(iq * 512, (iq + 1) * 512)
                for kk in range(2):
                    nc.sync.dma_start(
                        o8d_tile(pair)[:, kk, sl], o8f_tile(pair, kk)[:, sl])

            # ---------------- emission ----------------
            # prologue: pair0 q/k projections + reshape
            for ft in (0, 1):
                for ih in range(2):
                    qk_proj(ft, ih)
            qk_reshape(0, 0)
            qk_reshape(0, 1)
            v_proj(0)
            v_proj(1)
            v_proj(2)
            v_proj(3)

            # aux work queue: emitted interleaved under pair0 attention.
            # v_proj(jt) must be emitted before sums of unit t=jt//2.
            aux = [lambda n=nt: v_proj(n) for nt in range(4, 16)]
            for ft in (2, 3):
                for ih in range(2):
                    aux.append(lambda f=ft, i=ih: qk_proj(f, i))
            aux.append(lambda: qk_reshape(1, 0))
            aux.append(lambda: qk_reshape(1, 1))

            def pop_aux(k):
                for _ in range(k):
                    if aux:
                        aux.pop(0)()

            units = {}
            order = [(t, h) for t in range(8) for h in (0, 1)]
            TILES = [(0, 0), (0, 1), (1, 0), (1, 1)]

            def mk_groups(pair):
                us = [Unit(pair, t, h) for (t, h) in order]
                for u in us:
                    units[(pair, u.t, u.h)] = u
                aq = [u for u in us if u.eng == 'A']
                dq = [u for u in us if u.eng == 'D']
                gs = []
                while aq or dq:
                    g = []
                    if aq:
                        g.append(aq.pop(0))
                    if dq:
                        g.append(dq.pop(0))
                    gs.append(g)
                return gs

            state = {}

            def pair_quarters(pair, nxt_drain):
                """Emit quarters 1-3 of `pair` as a work queue: each item
                runs on its own lazily-allocated oh set; nxt_drain lets the
                caller interleave these under the next pair's groups."""
                q = []
                for iq in range(1, 4):
                    def alloc(p=pair, i=iq):
                        state[(p, i)] = oh_alloc()
                    q.append(alloc)
                    for t, h in order:
                        q.append(lambda p=pair, i=iq, tt=t, hh=h:
                                 units[(p, tt, hh)].pv(i, state[(p, i)]))
                    def fin(p=pair, i=iq):
                        o_evac(p, i, state[(p, i)])
                        o_fold(p, i)
                        nxt_drain.extend(
                            lambda c=cot, pp_=p, ii=i: out_proj(pp_, c, ii)
                            for cot in range(4))
                    q.append(fin)
                return q

            # ---- pair 0: interleaved groups, proj/v aux between ----
            oh0 = oh_alloc()
            pend = None
            for g in mk_groups(0):
                for kk, ih in TILES:
                    for u in g:
                        u.s_tile(kk, ih)
                if pend:
                    for u in pend:
                        u.pv(0, oh0)
                for u in g:
                    u.sums_and_vp()
                pend = g
                pop_aux(3)
            for u in pend:
                u.pv(0, oh0)
            while aux:
                pop_aux(1)
            o_evac(0, 0, oh0)
            o_fold(0, 0)
            q_op = [lambda c=cot: out_proj(0, c, 0) for cot in range(4)]
            q_p0 = pair_quarters(0, q_op)

            # ---- pair 1 groups, draining pair0 quarters + out-proj ----
            q_p1a = []

            def get_oh1():
                if state.get((1, 0)) is None:
                    state[(1, 0)] = oh_alloc()
                return state[(1, 0)]

            def drain(n_budget=6):
                n = 0
                while q_p0 and n < n_budget:
                    q_p0.pop(0)()
                    n += 1
                # pair1 pv0s only after ALL pair0 quarters: the oh-tag
                # rotation means an early (1,0) alloc would wedge PE's
                # wait queue behind pair0's last evac
                if not q_p0:
                    while q_p1a and n < n_budget + 2:
                        q_p1a.pop(0)()
                        n += 1
                    if not q_p1a and q_op:
                        q_op.pop(0)()

            for g in mk_groups(1):
                for kk, ih in TILES:
                    for u in g:
                        u.s_tile(kk, ih)
                for u in g:
                    u.sums_and_vp()
                    q_p1a.append(lambda p=u: p.pv(0, get_oh1()))
                drain()
            while q_p0 or q_p1a:
                drain()
            o_evac(1, 0, state[(1, 0)])
            o_fold(1, 0)
            q_op.extend(lambda c=cot: out_proj(1, c, 0) for cot in range(4))

            # ---- pair 1 quarters 1-3 + remaining out-proj tail ----
            q_p1 = pair_quarters(1, q_op)
            while q_p1:
                q_p1.pop(0)()
                if q_op:
                    q_op.pop(0)()
            while q_op:
                q_op.pop(0)()

    nc.compile()
    return nc


def get_nc():
    global _NC
    if _NC is None:
        _NC = _build_nc()
    return _NC


def core_inputs(x, Wp, bp, core):
    """Host-side shard prep for one core: b = core//2, g = core%2."""
    b, g = divmod(core, 2)
    E4 = ml_dtypes.float8_e4m3

    def to8(a):
        return np.ascontiguousarray(np.asarray(a, np.float32).astype(E4))

    xb = x[b]  # [C, N]
    # x8[p, kt, kk, n] = x[kt*128 + kk*64 + p, n]
    x8 = np.transpose(xb.reshape(4, 2, 64, N), (2, 0, 1, 3))

    # qk feature order: ftile ft = 2*pair + (0=q,1=k); within: hA d0-63, hB
    qidx = np.zeros((4, 128), np.int64)
    for pair in range(2):
        for which in range(2):
            ft = 2 * pair + which
            for lh in range(2):
                h = 4 * g + 2 * pair + lh
                base = h * 192 + which * 64
                qidx[ft, lh * 64:(lh + 1) * 64] = np.arange(base, base + 64)
    Wqk = Wp[qidx.reshape(-1)]            # [512 feat, C]
    # wqk8[p, kt, kk, ft, j] = Wqk[ft*128 + j, kt*128 + kk*64 + p]
    wqk8 = np.transpose(
        Wqk.reshape(4, 128, 4, 2, 64), (4, 2, 3, 0, 1))
    bqk = bp[qidx.reshape(-1)].reshape(4, 128).T  # [128, 4]

    vidx = np.concatenate([np.arange((4 * g + lh) * 192 + 128,
                                     (4 * g + lh) * 192 + 192)
                           for lh in range(4)])
    Wv = Wp[vidx]                          # [256, C]
    # wv8[p, kt, kk, f] = Wv[f, kt*128 + kk*64 + p]
    wv8 = np.transpose(Wv.reshape(256, 4, 2, 64), (3, 1, 2, 0))

    return {
        "x8": to8(x8),
        "wqk8": to8(wqk8),
        "bqk": np.ascontiguousarray(bqk.astype(np.float32)),
        "wv8": to8(wv8),
        "bpv": bp[vidx].astype(np.float16).reshape(1, 256),
        "ones": np.ones((1, 128), np.float16),
    }


def wo_inputs(Wo, core):
    g = core % 2
    E4 = ml_dtypes.float8_e4m3
    # wo8[p, pair, kk, c] = Wo[c, 256*g + pair*128 + kk*64 + p]
    Wog = Wo[:, 256 * g:256 * (g + 1)]     # [C, 256]
    wo8 = np.transpose(Wog.reshape(C, 2, 2, 64), (3, 1, 2, 0))
    return np.ascontiguousarray(np.asarray(wo8, np.float32).astype(E4))


def kernel(x, Wp, bp, Wo, bo):
    global LAST_RESULT
    x = np.asarray(x, dtype=np.float32)
    Wp = np.asarray(Wp, dtype=np.float32)
    bp = np.asarray(bp, dtype=np.float32)
    Wo = np.asarray(Wo, dtype=np.float32)
    bo = np.asarray(bo, dtype=np.float32)

    in_maps = []
    for core in range(N_CORES):
        m = core_inputs(x, Wp, bp, core)
        m["wo8"] = wo_inputs(Wo, core)
        in_maps.append(m)

    nc = get_nc()
    res = run_bass_kernel_spmd(
        nc, in_maps, core_ids=list(range(N_CORES)),
        trace=bool(int(os.environ.get("KERNEL_TRACE", "0"))),
    )
    LAST_RESULT = res
    result = np.empty((B, C, N), dtype=np.float32)
    for b in range(B):
        r0, r1 = res.results[2 * b], res.results[2 * b + 1]
        result[b] = (
            (r0["out_a"].astype(np.float32) + r0["out_b"].astype(np.float32)
             + r1["out_a"].astype(np.float32) + r1["out_b"].astype(np.float32))
            / HOST_DIV
            + x[b] + bo[:, None]
        )
    return result


# revision 38
# speedup vs baseline: 1.1854x; 1.0309x over previous
"""Trainium2 Bass kernel for nn_AttentionBlock (B=4, C=512, N=2048, H=8, DK=64).

Computation (see module docstring of the reference):
  xt = x.T; qkv = xt @ Wp.T + bp; per head: S[j,i] = k_j . q_i / 8,
  P = softmax over i (query axis => per-j rows of S^T), O = P^T-weighted
  v-mix, out = (O @ Wo.T + bo + xt).T.

Sharding: 8 cores = (batch b = core//2) x (head-group g = core%2, 4 heads).
Each core emits two f16 partial resT [C, N] tensors (one per head pair);
host sums partials (x8 scale), adds bias + residual.

Engine strategy (cost-model driven):
  - All heavy matmuls are fp8e4 DoubleRow (0.5 cyc/row): QK/V projections
    (host supplies x/W pre-packed [64, 2, .] k-tiles), S (q/k re-packed to
    [32, 2, N] via SBUF->SBUF DMA), PV (contracts jt PAIRS: K=256 as
    [128, 2, .]), out-projection (o folded to [64, 2, N] via DMA).
  - exp work is split across THREE engines per (jtpair, head) unit:
      ACT: native Exp (scale=1/8, bias=-2ln2 so E<=61 fits e4m3),
           fp8 E tiles + free accum_out row sums -> DR PV.
      DVE/Pool: Schraudolph bit-trick exp: i16 = floor(A*S + B16) is the
           bit pattern of f16(~exp(S/8)); sums via a DVE tensor_scalar
           accum pass (4x mode); PV in fp16 for these tiles.
  - Normalization folds into v: vp = v * (VP/sumE); PV accumulates
    VP-scaled O in PSUM; o-evac rescales by 8/VP into fp8; host /8.
  - PSUM: 2 rotating [128,1024] S-slots (ACT + DVE consumers) + two
    per-head [64,1024] O-half accumulators (DR matmuls cannot write at a
    partition offset); PV runs in two i-half passes.
"""

import math
import os
import numpy as np
import ml_dtypes

import concourse.bass as bass
import concourse.tile as tile
from concourse import bacc, mybir
from concourse.bass_utils import run_bass_kernel_spmd

F32 = mybir.dt.float32
F16 = mybir.dt.float16
F8 = mybir.dt.float8e4
I16 = mybir.dt.int16
AF = mybir.ActivationFunctionType
ALU = mybir.AluOpType
DR = mybir.MatmulPerfMode.DoubleRow

B, C, N = 4, 512, 2048
H, DK = 8, 64
N_CORES = 8
SCALE = DK ** -0.5              # 0.125
EXPB = -2.0 * math.log(2.0)     # ACT exp bias: E' = exp(z)/4 (max ~61 < 240)
A_SCH = 1024.0 * 1.4426950408889634 * SCALE   # Schraudolph slope on raw S
B_SCH = 15294.0                 # tuned offset (incl trunc+centering)
VP = 512.0                      # vp pre-scale (fp8 precision for v/sumE)
OEV = 8.0 / VP                  # o-evac rescale: o8 = 8*O_true; host /8
HOST_DIV = 8.0

# per (pair, jtpair t, head h): engine for the exp unit.
# 'A' = ACT native exp (fp8 E, DR PV); 'D' = DVE Schraudolph (f16 E, fp16
# PV). GPSIMD cannot read PSUM (BIR verifier), so Pool only gets the
# SBUF-side work: sum passes over f16 E tiles, vp scaling, ssum adds.
ASSIGN = [
    [('A','A'),('A','D'),('D','A'),('A','D'),('D','A'),('A','D'),('D','A'),('A','A')],
    [('A','A'),('A','D'),('D','A'),('A','D'),('D','A'),('A','D'),('D','A'),('A','A')],
]

LAST_RESULT = None
_NC = None


def _build_nc():
    nc = bacc.Bacc("TRN2", target_bir_lowering=False, debug=False,
                   num_devices=N_CORES)

    x8 = nc.dram_tensor("x8", [64, 4, 2, N], F8, kind="ExternalInput").ap()
    wqk8 = nc.dram_tensor("wqk8", [64, 4, 2, 4, 128], F8, kind="ExternalInput").ap()
    bqk = nc.dram_tensor("bqk", [128, 4], F32, kind="ExternalInput").ap()
    wv8 = nc.dram_tensor("wv8", [64, 4, 2, 256], F8, kind="ExternalInput").ap()
    bpv = nc.dram_tensor("bpv", [1, 256], F16, kind="ExternalInput").ap()
    wo8 = nc.dram_tensor("wo8", [64, 2, 2, C], F8, kind="ExternalInput").ap()
    ones = nc.dram_tensor("ones", [1, 128], F16, kind="ExternalInput").ap()
    out_a = nc.dram_tensor("out_a", [C, N], F16, kind="ExternalOutput").ap()
    out_b = nc.dram_tensor("out_b", [C, N], F16, kind="ExternalOutput").ap()

    with tile.TileContext(nc) as tc:
        with (
            tc.tile_pool(name="consts", bufs=1) as consts,
            tc.tile_pool(name="qkpool", bufs=1) as qkpool,
            tc.tile_pool(name="vpool", bufs=1) as vpool,
            tc.tile_pool(name="e8pool", bufs=11) as e8pool,
            tc.tile_pool(name="e16pool", bufs=7) as e16pool,
            tc.tile_pool(name="vppool", bufs=18) as vppool,
            tc.tile_pool(name="opool", bufs=2) as opool,
            tc.tile_pool(name="outpool", bufs=2) as outpool,
            tc.tile_pool(name="smalls", bufs=40) as smalls,
            tc.tile_pool(name="psum", bufs=1, space="PSUM") as pp,
        ):
            ones_sb = consts.tile([1, 128], F16)
            nc.sync.dma_start(ones_sb[:], ones[:])
            bqk_sb = consts.tile([128, 4], F32)
            nc.sync.dma_start(bqk_sb[:], bqk[:])
            bpv_sb = consts.tile([1, 256], F16)
            nc.sync.dma_start(bpv_sb[:], bpv[:])
            wqk_sb = consts.tile([64, 4, 2, 4, 128], F8)
            nc.sync.dma_start(wqk_sb[:], wqk8[:])
            x_kt = []
            for kt in range(4):
                t_ = consts.tile([64, 2, N], F8, name=f"x_kt{kt}")
                nc.sync.dma_start(t_[:], x8[:, kt])
                x_kt.append(t_)
            wv_sb = consts.tile([64, 4, 2, 256], F8)
            nc.sync.dma_start(wv_sb[:], wv8[:])
            wo_sb = consts.tile([64, 2, 2, C], F8)
            nc.sync.dma_start(wo_sb[:], wo8[:])

            # warm the ACT exp table while DMAs run
            warm = smalls.tile([1, 128], F16, tag="warm", bufs=1, name="warm")
            nc.scalar.activation(warm[:], ones_sb[:], AF.Exp)
            expb_sb = consts.tile([128, 1], F32)
            nc.vector.memset(expb_sb[:], EXPB)
            zrow_sb = consts.tile([1, 512], F16)
            nc.vector.memset(zrow_sb[:], 0.0)

            # persistent SBUF tensors
            # qk8: fp8 evac of the QK projection [128 feat, N], rotated
            qk8_of = {}

            def qk8_tile(ft):
                if ft not in qk8_of:
                    qk8_of[ft] = qkpool.tile([128, N], F8, tag="qk8e",
                                             bufs=2, name="qk8e")
                return qk8_of[ft]
            # qh8: S-DR layout [32, (head,qk) 8, kk 2, N]
            qh8 = qkpool.tile([32, 8, 2, N], F8, name="qh8")
            v16 = vpool.tile([128, 16, 256], F16, name="v16")
            # o8f: per-pair, per-head fp8 o evac [64 d, N] (partition 0-63)
            o8f_of = {}

            def o8f_tile(p, h):
                if (p, h) not in o8f_of:
                    o8f_of[(p, h)] = opool.tile([64, N], F8, tag=f"o8f{h}",
                                                bufs=1, name="o8f")
                return o8f_of[(p, h)]
            # o8d: DR-folded [64, 2, N]
            o8d_of = {}

            def o8d_tile(p):
                if p not in o8d_of:
                    o8d_of[p] = opool.tile([64, 2, N], F8, tag="o8d", bufs=1,
                                           name="o8d")
                return o8d_of[p]
            scr16 = qkpool.tile([128, 2048], F16, name="scr16")

            def s_slot():
                # ACT-dedicated slots (also used by proj/outproj fills)
                return pp.tile([128, 1024], F32, tag="s", bufs=2, name="s_ps")

            def sd_slot():
                # DVE-dedicated half-size slots: decouples the DVE consumer
                # chain from ACT's so neither stalls the other
                return pp.tile([128, 512], F32, tag="sd", bufs=2,
                               name="sd_ps")

            def qk_proj(ft, ih):
                # one i-half of ftile ft -> PSUM -> fp8 evac into qk8[:, ft]
                ps = s_slot()
                for ic in range(4):
                    nch = ih * 1024 + ic * 256
                    for kt in range(4):
                        nc.tensor.matmul(
                            ps[:, ic * 256:(ic + 1) * 256],
                            lhsT=wqk_sb[:, kt, :, ft, :],
                            rhs=x_kt[kt][:, :, nch:nch + 256],
                            start=(kt == 0), stop=(kt == 3), perf_mode=DR,
                        )
                nc.scalar.activation(
                    qk8_tile(ft)[:, ih * 1024:(ih + 1) * 1024],
                    ps[:], AF.Identity, bias=bqk_sb[:, ft:ft + 1])

            def qk_reshape(pair, which):
                # fold [128, N] ftile into S-DR layout [32, 2, N] per head
                ft = 2 * pair + which
                src = qk8_tile(ft)
                for lh in range(2):
                    ht = (2 * pair + lh) * 2 + which
                    for kk in range(2):
                        base = lh * 64 + kk * 32
                        nc.sync.dma_start(
                            qh8[:, ht, kk, :], src[base:base + 32, :])

            def v_proj(nt):
                ps = s_slot()
                for kt in range(4):
                    nc.tensor.matmul(
                        ps[:, :256],
                        lhsT=x_kt[kt][:, :, nt * 128:(nt + 1) * 128],
                        rhs=wv_sb[:, kt], start=(kt == 0), stop=False,
                        perf_mode=DR,
                    )
                nc.tensor.matmul(
                    ps[:, :256], lhsT=ones_sb[:1, :], rhs=bpv_sb[:1, :],
                    start=False, stop=True,
                )
                nc.scalar.copy(v16[:, nt, :], ps[:, :256])

            op_count = [0]

            def out_proj(pair, cot, iq):
                # [128 c, 512 i] DR out-proj chunk + f16 evac + DMA
                dst = out_a if pair == 0 else out_b
                ps = s_slot()
                for ic in range(2):
                    i0 = iq * 512 + ic * 256
                    nc.tensor.matmul(
                        ps[:, ic * 256:(ic + 1) * 256],
                        lhsT=wo_sb[:, pair, :, cot * 128:(cot + 1) * 128],
                        rhs=o8d_tile(pair)[:, :, i0:i0 + 256],
                        start=True, stop=True, perf_mode=DR,
                    )
                ot = outpool.tile([128, 512], F16, tag="out", bufs=4,
                                  name="out_t")
                eng = [nc.scalar.copy,
                       nc.vector.tensor_copy][op_count[0] % 2]
                op_count[0] += 1
                eng(ot[:], ps[:, :512])
                nc.sync.dma_start(
                    dst[cot * 128:(cot + 1) * 128,
                        iq * 512:(iq + 1) * 512], ot[:])

            class Unit:
                """One (pair, jtpair, head) exp unit."""

                def __init__(self, pair, t, h):
                    self.pair, self.t, self.h = pair, t, h
                    self.eng = ASSIGN[pair][t][h]
                    self.ht_q = (2 * pair + h) * 2
                    self.ht_k = self.ht_q + 1
                    if self.eng == 'A':
                        self.e8 = e8pool.tile([128, 2, N], F8, tag="e8",
                                              name="e8")
                    else:
                        self.e16 = e16pool.tile([128, 2, N], I16, tag="e16",
                                                name="e16")
                    self.accs = {}
                    self.recs = {}
                    self.vps = None

                def s_and_consume(self):
                    # 4 PSUM tiles: (kk=jt-of-pair, ih=i-half)
                    for kk in range(2):
                        for ih in range(2):
                            self.s_tile(kk, ih)

                def s_tile(self, kk, ih):
                    # ACT: one [128 j, 1024 i] fill + exp consumer.
                    # DVE: ih indexes QUARTERS [128, 512] (8 tiles/unit).
                    jt = 2 * self.t + kk
                    if self.eng == 'A':
                        ps = s_slot()
                        for ic in range(4):
                            i0 = ih * 1024 + ic * 256
                            nc.tensor.matmul(
                                ps[:, ic * 256:(ic + 1) * 256],
                                lhsT=qh8[:, self.ht_k, :,
                                         jt * 128:(jt + 1) * 128],
                                rhs=qh8[:, self.ht_q, :, i0:i0 + 256],
                                start=True, stop=True, perf_mode=DR,
                            )
                        acc = smalls.tile([128, 1], F32, tag="acc",
                                          bufs=12, name="acc")
                        nc.scalar.activation(
                            self.e8[:, kk, ih * 1024:(ih + 1) * 1024],
                            ps[:], AF.Exp, scale=SCALE,
                            bias=expb_sb[:], accum_out=acc,
                        )
                        self.accs[(kk, ih)] = acc
                    else:
                        for q in range(2):
                            ps = sd_slot()
                            for ic in range(2):
                                i0 = ih * 1024 + q * 512 + ic * 256
                                nc.tensor.matmul(
                                    ps[:, ic * 256:(ic + 1) * 256],
                                    lhsT=qh8[:, self.ht_k, :,
                                             jt * 128:(jt + 1) * 128],
                                    rhs=qh8[:, self.ht_q, :, i0:i0 + 256],
                                    start=True, stop=True, perf_mode=DR,
                                )
                            i0 = ih * 1024 + q * 512
                            nc.vector.tensor_scalar(
                                self.e16[:, kk, i0:i0 + 512],
                                ps[:], A_SCH, B_SCH, ALU.mult, ALU.add)

                def sums_and_vp(self):
                    vps = vppool.tile([128, 2, 64], F8 if self.eng == 'A'
                                      else F16, tag="vp", name="vp")
                    self.vps = vps
                    for kk in range(2):
                        jt = 2 * self.t + kk
                        ssum = smalls.tile([128, 1], F32, tag="ssum", bufs=12,
                                           name="ssum")
                        if self.eng == 'A':
                            nc.gpsimd.tensor_add(
                                ssum[:], self.accs[(kk, 0)][:],
                                self.accs[(kk, 1)][:])
                        else:
                            nc.vector.tensor_scalar(
                                scr16[:], self.e16[:, kk, :].bitcast(F16),
                                1.0, None, ALU.mult, ALU.add,
                                accum_out=ssum[:])
                        rec = smalls.tile([128, 1], F32, tag="rec", bufs=12,
                                          name="rec")
                        nc.vector.reciprocal(rec[:], ssum[:])
                        lv = 2 * self.pair + self.h
                        nc.gpsimd.tensor_scalar(
                            vps[:, kk, :],
                            v16[:, jt, lv * 64:(lv + 1) * 64],
                            rec[:], VP, ALU.mult, ALU.mult,
                        )

                def pv(self, iq, ohs):
                    stop = (self.t == 7)
                    o_ps = ohs[self.h]
                    if self.eng == 'A':
                        for ic in range(2):
                            i0 = iq * 512 + ic * 256
                            nc.tensor.matmul(
                                o_ps[:, ic * 256:(ic + 1) * 256],
                                lhsT=self.vps[:],
                                rhs=self.e8[:, :, i0:i0 + 256],
                                start=False, stop=stop, perf_mode=DR,
                                skip_group_check=True,
                            )
                    else:
                        ef = self.e16[:].bitcast(F16)
                        i0 = iq * 512
                        for kk in range(2):
                            nc.tensor.matmul(
                                o_ps[:],
                                lhsT=self.vps[:, kk, :],
                                rhs=ef[:, kk, i0:i0 + 512],
                                start=False, stop=(stop and kk == 1),
                                skip_group_check=True,
                            )

            def oh_alloc():
                # fresh per-head O-QUARTER accumulators [64, 512] (partition
                # 0-63: DR matmuls cannot target a partition offset).
                # zero-matmuls set has_written so every PV is an accumulate.
                ohs = []
                for h in range(2):
                    o_ps = pp.tile([64, 512], F32, tag=f"oh{h}", bufs=1,
                                   name=f"o_ps{h}")
                    nc.tensor.matmul(
                        o_ps[:], lhsT=zrow_sb[:1, :64], rhs=zrow_sb[:1, :],
                        start=True, stop=False, skip_group_check=True,
                    )
                    ohs.append(o_ps)
                return ohs

            def o_evac(pair, iq, ohs):
                for h in range(2):
                    nc.vector.tensor_scalar(
                        o8f_tile(pair, h)[:, iq * 512:(iq + 1) * 512],
                        ohs[h][:], OEV, None, ALU.mult)

            def o_fold(pair, iq):
                sl = slice(iq * 512, (iq + 1) * 512)
                for kk in range(2):
                    nc.sync.dma_start(
                        o8d_tile(pair)[:, kk, sl], o8f_tile(pair, kk)[:, sl])

            # ---------------- emission ----------------
            # prologue: pair0 q/k projections + reshape
            for ft in (0, 1):
                for ih in range(2):
                    qk_proj(ft, ih)
            qk_reshape(0, 0)
            qk_reshape(0, 1)
            v_proj(0)
            v_proj(1)
            v_proj(2)
            v_proj(3)

            # aux work queue: emitted interleaved under pair0 attention.
            # v_proj(jt) must be emitted before sums of unit t=jt//2.
            aux = [lambda n=nt: v_proj(n) for nt in range(4, 16)]
            for ft in (2, 3):
                for ih in range(2):
                    aux.append(lambda f=ft, i=ih: qk_proj(f, i))
            aux.append(lambda: qk_reshape(1, 0))
            aux.append(lambda: qk_reshape(1, 1))

            def pop_aux(k):
                for _ in range(k):
                    if aux:
                        aux.pop(0)()

            units = {}
            order = [(t, h) for t in range(8) for h in (0, 1)]
            TILES = [(0, 0), (0, 1), (1, 0), (1, 1)]

            def mk_groups(pair):
                us = [Unit(pair, t, h) for (t, h) in order]
                for u in us:
                    units[(pair, u.t, u.h)] = u
                aq = [u for u in us if u.eng == 'A']
                dq = [u for u in us if u.eng == 'D']
                gs = []
                while aq or dq:
                    g = []
                    if aq:
                        g.append(aq.pop(0))
                    if dq:
                        g.append(dq.pop(0))
                    gs.append(g)
                return gs

            state = {}

            def pair_quarters(pair, nxt_drain):
                """Emit quarters 1-3 of `pair` as a work queue: each item
                runs on its own lazily-allocated oh set; nxt_drain lets the
                caller interleave these under the next pair's groups."""
                q = []
                for iq in range(1, 4):
                    def alloc(p=pair, i=iq):
                        state[(p, i)] = oh_alloc()
                    q.append(alloc)
                    for t, h in order:
                        q.append(lambda p=pair, i=iq, tt=t, hh=h:
                                 units[(p, tt, hh)].pv(i, state[(p, i)]))
                    def fin(p=pair, i=iq):
                        o_evac(p, i, state[(p, i)])
                        o_fold(p, i)
                        nxt_drain.extend(
                            lambda c=cot, pp_=p, ii=i: out_proj(pp_, c, ii)
                            for cot in range(4))
                    q.append(fin)
                return q

            # ---- pair 0: interleaved groups, proj/v aux between ----
            oh0 = oh_alloc()
            pend = None
            for g in mk_groups(0):
                for kk, ih in TILES:
                    for u in g:
                        u.s_tile(kk, ih)
                if pend:
                    for u in pend:
                        u.pv(0, oh0)
                for u in g:
                    u.sums_and_vp()
                pend = g
                pop_aux(3)
            for u in pend:
                u.pv(0, oh0)
            while aux:
                pop_aux(1)
            o_evac(0, 0, oh0)
            o_fold(0, 0)
            q_op = [lambda c=cot: out_proj(0, c, 0) for cot in range(4)]
            q_p0 = pair_quarters(0, q_op)

            # ---- pair 1 groups, draining pair0 quarters + out-proj ----
            q_p1a = []

            def get_oh1():
                if state.get((1, 0)) is None:
                    state[(1, 0)] = oh_alloc()
                return state[(1, 0)]

            def drain(n_budget=6):
                n = 0
                while q_p0 and n < n_budget:
                    q_p0.pop(0)()
                    n += 1
                # pair1 pv0s only after ALL pair0 quarters: the oh-tag
                # rotation means an early (1,0) alloc would wedge PE's
                # wait queue behind pair0's last evac
                if not q_p0:
                    while q_p1a and n < n_budget + 2:
                        q_p1a.pop(0)()
                        n += 1
                    if not q_p1a and q_op:
                        q_op.pop(0)()

            for g in mk_groups(1):
                for kk, ih in TILES:
                    for u in g:
                        u.s_tile(kk, ih)
                for u in g:
                    u.sums_and_vp()
                    q_p1a.append(lambda p=u: p.pv(0, get_oh1()))
                drain()
            while q_p0 or q_p1a:
                drain()
            o_evac(1, 0, state[(1, 0)])
            o_fold(1, 0)
            q_op.extend(lambda c=cot: out_proj(1, c, 0) for cot in range(4))

            # ---- pair 1 quarters 1-3 + remaining out-proj tail ----
            q_p1 = pair_quarters(1, q_op)
            while q_p1:
                q_p1.pop(0)()
                if q_op:
                    q_op.pop(0)()
            while q_op:
                q_op.pop(0)()

    nc.compile()
    return nc


def get_nc():
    global _NC
    if _NC is None:
        _NC = _build_nc()
    return _NC


def core_inputs(x, Wp, bp, core):
    """Host-side shard prep for one core: b = core//2, g = core%2."""
    b, g = divmod(core, 2)
    E4 = ml_dtypes.float8_e4m3

    def to8(a):
        return np.ascontiguousarray(np.asarray(a, np.float32).astype(E4))

    xb = x[b]  # [C, N]
    # x8[p, kt, kk, n] = x[kt*128 + kk*64 + p, n]
    x8 = np.transpose(xb.reshape(4, 2, 64, N), (2, 0, 1, 3))

    # qk feature order: ftile ft = 2*pair + (0=q,1=k); within: hA d0-63, hB
    qidx = np.zeros((4, 128), np.int64)
    for pair in range(2):
        for which in range(2):
            ft = 2 * pair + which
            for lh in range(2):
                h = 4 * g + 2 * pair + lh
                base = h * 192 + which * 64
                qidx[ft, lh * 64:(lh + 1) * 64] = np.arange(base, base + 64)
    Wqk = Wp[qidx.reshape(-1)]            # [512 feat, C]
    # wqk8[p, kt, kk, ft, j] = Wqk[ft*128 + j, kt*128 + kk*64 + p]
    wqk8 = np.transpose(
        Wqk.reshape(4, 128, 4, 2, 64), (4, 2, 3, 0, 1))
    bqk = bp[qidx.reshape(-1)].reshape(4, 128).T  # [128, 4]

    vidx = np.concatenate([np.arange((4 * g + lh) * 192 + 128,
                                     (4 * g + lh) * 192 + 192)
                           for lh in range(4)])
    Wv = Wp[vidx]                          # [256, C]
    # wv8[p, kt, kk, f] = Wv[f, kt*128 + kk*64 + p]
    wv8 = np.transpose(Wv.reshape(256, 4, 2, 64), (3, 1, 2, 0))

    return {
        "x8": to8(x8),
        "wqk8": to8(wqk8),
        "bqk": np.ascontiguousarray(bqk.astype(np.float32)),
        "wv8": to8(wv8),
        "bpv": bp[vidx].astype(np.float16).reshape(1, 256),
        "ones": np.ones((1, 128), np.float16),
    }


def wo_inputs(Wo, core):
    g = core % 2
    E4 = ml_dtypes.float8_e4m3
    # wo8[p, pair, kk, c] = Wo[c, 256*g + pair*128 + kk*64 + p]
    Wog = Wo[:, 256 * g:256 * (g + 1)]     # [C, 256]
    wo8 = np.transpose(Wog.reshape(C, 2, 2, 64), (3, 1, 2, 0))
    return np.ascontiguousarray(np.asarray(wo8, np.float32).astype(E4))


def kernel(x, Wp, bp, Wo, bo):
    global LAST_RESULT
    x = np.asarray(x, dtype=np.float32)
    Wp = np.asarray(Wp, dtype=np.float32)
    bp = np.asarray(bp, dtype=np.float32)
    Wo = np.asarray(Wo, dtype=np.float32)
    bo = np.asarray(bo, dtype=np.float32)

    in_maps = []
    for core in range(N_CORES):
        m = core_inputs(x, Wp, bp, core)
        m["wo8"] = wo_inputs(Wo, core)
        in_maps.append(m)

    nc = get_nc()
    res = run_bass_kernel_spmd(
        nc, in_maps, core_ids=list(range(N_CORES)),
        trace=bool(int(os.environ.get("KERNEL_TRACE", "0"))),
    )
    LAST_RESULT = res
    result = np.empty((B, C, N), dtype=np.float32)
    for b in range(B):
        r0, r1 = res.results[2 * b], res.results[2 * b + 1]
        result[b] = (
            (r0["out_a"].astype(np.float32) + r0["out_b"].astype(np.float32)
             + r1["out_a"].astype(np.float32) + r1["out_b"].astype(np.float32))
            / HOST_DIV
            + x[b] + bo[:, None]
        )
    return result


# revision 39
# speedup vs baseline: 1.1921x; 1.0056x over previous
"""Trainium2 Bass kernel for nn_AttentionBlock (B=4, C=512, N=2048, H=8, DK=64).

Computation (see module docstring of the reference):
  xt = x.T; qkv = xt @ Wp.T + bp; per head: S[j,i] = k_j . q_i / 8,
  P = softmax over i (query axis => per-j rows of S^T), O = P^T-weighted
  v-mix, out = (O @ Wo.T + bo + xt).T.

Sharding: 8 cores = (batch b = core//2) x (head-group g = core%2, 4 heads).
Each core emits two f16 partial resT [C, N] tensors (one per head pair);
host sums partials (x8 scale), adds bias + residual.

Engine strategy (cost-model driven):
  - All heavy matmuls are fp8e4 DoubleRow (0.5 cyc/row): QK/V projections
    (host supplies x/W pre-packed [64, 2, .] k-tiles), S (q/k re-packed to
    [32, 2, N] via SBUF->SBUF DMA), PV (contracts jt PAIRS: K=256 as
    [128, 2, .]), out-projection (o folded to [64, 2, N] via DMA).
  - exp work is split across THREE engines per (jtpair, head) unit:
      ACT: native Exp (scale=1/8, bias=-2ln2 so E<=61 fits e4m3),
           fp8 E tiles + free accum_out row sums -> DR PV.
      DVE/Pool: Schraudolph bit-trick exp: i16 = floor(A*S + B16) is the
           bit pattern of f16(~exp(S/8)); sums via a DVE tensor_scalar
           accum pass (4x mode); PV in fp16 for these tiles.
  - Normalization folds into v: vp = v * (VP/sumE); PV accumulates
    VP-scaled O in PSUM; o-evac rescales by 8/VP into fp8; host /8.
  - PSUM: 2 rotating [128,1024] S-slots (ACT + DVE consumers) + two
    per-head [64,1024] O-half accumulators (DR matmuls cannot write at a
    partition offset); PV runs in two i-half passes.
"""

import math
import os
import numpy as np
import ml_dtypes

import concourse.bass as bass
import concourse.tile as tile
from concourse import bacc, mybir
from concourse.bass_utils import run_bass_kernel_spmd

F32 = mybir.dt.float32
F16 = mybir.dt.float16
F8 = mybir.dt.float8e4
I16 = mybir.dt.int16
AF = mybir.ActivationFunctionType
ALU = mybir.AluOpType
DR = mybir.MatmulPerfMode.DoubleRow

B, C, N = 4, 512, 2048
H, DK = 8, 64
N_CORES = 8
SCALE = DK ** -0.5              # 0.125
EXPB = -2.0 * math.log(2.0)     # ACT exp bias: E' = exp(z)/4 (max ~61 < 240)
A_SCH = 1024.0 * 1.4426950408889634 * SCALE   # Schraudolph slope on raw S
B_SCH = 15294.0                 # tuned offset (incl trunc+centering)
VP = 512.0                      # vp pre-scale (fp8 precision for v/sumE)
OEV = 8.0 / VP                  # o-evac rescale: o8 = 8*O_true; host /8
HOST_DIV = 8.0

# per (pair, jtpair t, head h): engine for the exp unit.
# 'A' = ACT native exp (fp8 E, DR PV); 'D' = DVE Schraudolph (f16 E, fp16
# PV). GPSIMD cannot read PSUM (BIR verifier), so Pool only gets the
# SBUF-side work: sum passes over f16 E tiles, vp scaling, ssum adds.
ASSIGN = [
    [('A','A'),('A','D'),('D','A'),('A','D'),('D','A'),('A','D'),('D','D'),('A','A')],
    [('A','A'),('A','D'),('D','A'),('A','D'),('D','A'),('A','D'),('D','A'),('A','A')],
]

LAST_RESULT = None
_NC = None


def _build_nc():
    nc = bacc.Bacc("TRN2", target_bir_lowering=False, debug=False,
                   num_devices=N_CORES)

    x8 = nc.dram_tensor("x8", [64, 4, 2, N], F8, kind="ExternalInput").ap()
    wqk8 = nc.dram_tensor("wqk8", [64, 4, 2, 4, 128], F8, kind="ExternalInput").ap()
    bqk = nc.dram_tensor("bqk", [128, 4], F32, kind="ExternalInput").ap()
    wv8 = nc.dram_tensor("wv8", [64, 4, 2, 256], F8, kind="ExternalInput").ap()
    bpv = nc.dram_tensor("bpv", [1, 256], F16, kind="ExternalInput").ap()
    wo8 = nc.dram_tensor("wo8", [64, 2, 2, C], F8, kind="ExternalInput").ap()
    ones = nc.dram_tensor("ones", [1, 128], F16, kind="ExternalInput").ap()
    out_a = nc.dram_tensor("out_a", [C, N], F16, kind="ExternalOutput").ap()
    out_b = nc.dram_tensor("out_b", [C, N], F16, kind="ExternalOutput").ap()

    with tile.TileContext(nc) as tc:
        with (
            tc.tile_pool(name="consts", bufs=1) as consts,
            tc.tile_pool(name="qkpool", bufs=1) as qkpool,
            tc.tile_pool(name="vpool", bufs=1) as vpool,
            tc.tile_pool(name="e8pool", bufs=11) as e8pool,
            tc.tile_pool(name="e16pool", bufs=7) as e16pool,
            tc.tile_pool(name="vppool", bufs=18) as vppool,
            tc.tile_pool(name="opool", bufs=2) as opool,
            tc.tile_pool(name="outpool", bufs=2) as outpool,
            tc.tile_pool(name="smalls", bufs=40) as smalls,
            tc.tile_pool(name="psum", bufs=1, space="PSUM") as pp,
        ):
            ones_sb = consts.tile([1, 128], F16)
            nc.sync.dma_start(ones_sb[:], ones[:])
            bqk_sb = consts.tile([128, 4], F32)
            nc.sync.dma_start(bqk_sb[:], bqk[:])
            bpv_sb = consts.tile([1, 256], F16)
            nc.sync.dma_start(bpv_sb[:], bpv[:])
            wqk_sb = consts.tile([64, 4, 2, 4, 128], F8)
            nc.sync.dma_start(wqk_sb[:], wqk8[:])
            x_kt = []
            for kt in range(4):
                t_ = consts.tile([64, 2, N], F8, name=f"x_kt{kt}")
                nc.sync.dma_start(t_[:], x8[:, kt])
                x_kt.append(t_)
            wv_sb = consts.tile([64, 4, 2, 256], F8)
            nc.sync.dma_start(wv_sb[:], wv8[:])
            wo_sb = consts.tile([64, 2, 2, C], F8)
            nc.sync.dma_start(wo_sb[:], wo8[:])

            # warm the ACT exp table while DMAs run
            warm = smalls.tile([1, 128], F16, tag="warm", bufs=1, name="warm")
            nc.scalar.activation(warm[:], ones_sb[:], AF.Exp)
            expb_sb = consts.tile([128, 1], F32)
            nc.vector.memset(expb_sb[:], EXPB)
            zrow_sb = consts.tile([1, 512], F16)
            nc.vector.memset(zrow_sb[:], 0.0)

            # persistent SBUF tensors
            # qk8: fp8 evac of the QK projection [128 feat, N], rotated
            qk8_of = {}

            def qk8_tile(ft):
                if ft not in qk8_of:
                    qk8_of[ft] = qkpool.tile([128, N], F8, tag="qk8e",
                                             bufs=2, name="qk8e")
                return qk8_of[ft]
            # qh8: S-DR layout [32, (head,qk) 8, kk 2, N]
            qh8 = qkpool.tile([32, 8, 2, N], F8, name="qh8")
            v16 = vpool.tile([128, 16, 256], F16, name="v16")
            # o8f: per-pair, per-head fp8 o evac [64 d, N] (partition 0-63)
            o8f_of = {}

            def o8f_tile(p, h):
                if (p, h) not in o8f_of:
                    o8f_of[(p, h)] = opool.tile([64, N], F8, tag=f"o8f{h}",
                                                bufs=1, name="o8f")
                return o8f_of[(p, h)]
            # o8d: DR-folded [64, 2, N]
            o8d_of = {}

            def o8d_tile(p):
                if p not in o8d_of:
                    o8d_of[p] = opool.tile([64, 2, N], F8, tag="o8d", bufs=1,
                                           name="o8d")
                return o8d_of[p]
            scr16 = qkpool.tile([128, 2048], F16, name="scr16")

            def s_slot():
                # ACT-dedicated slots (also used by proj/outproj fills)
                return pp.tile([128, 1024], F32, tag="s", bufs=2, name="s_ps")

            def sd_slot():
                # DVE-dedicated half-size slots: decouples the DVE consumer
                # chain from ACT's so neither stalls the other
                return pp.tile([128, 512], F32, tag="sd", bufs=2,
                               name="sd_ps")

            def qk_proj(ft, ih):
                # one i-half of ftile ft -> PSUM -> fp8 evac into qk8[:, ft]
                ps = s_slot()
                for ic in range(4):
                    nch = ih * 1024 + ic * 256
                    for kt in range(4):
                        nc.tensor.matmul(
                            ps[:, ic * 256:(ic + 1) * 256],
                            lhsT=wqk_sb[:, kt, :, ft, :],
                            rhs=x_kt[kt][:, :, nch:nch + 256],
                            start=(kt == 0), stop=(kt == 3), perf_mode=DR,
                        )
                nc.scalar.activation(
                    qk8_tile(ft)[:, ih * 1024:(ih + 1) * 1024],
                    ps[:], AF.Identity, bias=bqk_sb[:, ft:ft + 1])

            def qk_reshape(pair, which):
                # fold [128, N] ftile into S-DR layout [32, 2, N] per head
                ft = 2 * pair + which
                src = qk8_tile(ft)
                for lh in range(2):
                    ht = (2 * pair + lh) * 2 + which
                    for kk in range(2):
                        base = lh * 64 + kk * 32
                        nc.sync.dma_start(
                            qh8[:, ht, kk, :], src[base:base + 32, :])

            def v_proj(nt):
                ps = s_slot()
                for kt in range(4):
                    nc.tensor.matmul(
                        ps[:, :256],
                        lhsT=x_kt[kt][:, :, nt * 128:(nt + 1) * 128],
                        rhs=wv_sb[:, kt], start=(kt == 0), stop=False,
                        perf_mode=DR,
                    )
                nc.tensor.matmul(
                    ps[:, :256], lhsT=ones_sb[:1, :], rhs=bpv_sb[:1, :],
                    start=False, stop=True,
                )
                if nt % 2 == 0:
                    nc.scalar.copy(v16[:, nt, :], ps[:, :256])
                else:
                    nc.vector.tensor_copy(v16[:, nt, :], ps[:, :256])

            op_count = [0]

            def out_proj(pair, cot, iq):
                # [128 c, 512 i] DR out-proj chunk + f16 evac + DMA
                dst = out_a if pair == 0 else out_b
                ps = s_slot()
                for ic in range(2):
                    i0 = iq * 512 + ic * 256
                    nc.tensor.matmul(
                        ps[:, ic * 256:(ic + 1) * 256],
                        lhsT=wo_sb[:, pair, :, cot * 128:(cot + 1) * 128],
                        rhs=o8d_tile(pair)[:, :, i0:i0 + 256],
                        start=True, stop=True, perf_mode=DR,
                    )
                ot = outpool.tile([128, 512], F16, tag="out", bufs=4,
                                  name="out_t")
                eng = [nc.scalar.copy,
                       nc.vector.tensor_copy][op_count[0] % 2]
                op_count[0] += 1
                eng(ot[:], ps[:, :512])
                nc.sync.dma_start(
                    dst[cot * 128:(cot + 1) * 128,
                        iq * 512:(iq + 1) * 512], ot[:])

            class Unit:
                """One (pair, jtpair, head) exp unit."""

                def __init__(self, pair, t, h):
                    self.pair, self.t, self.h = pair, t, h
                    self.eng = ASSIGN[pair][t][h]
                    self.ht_q = (2 * pair + h) * 2
                    self.ht_k = self.ht_q + 1
                    if self.eng == 'A':
                        self.e8 = e8pool.tile([128, 2, N], F8, tag="e8",
                                              name="e8")
                    else:
                        self.e16 = e16pool.tile([128, 2, N], I16, tag="e16",
                                                name="e16")
                    self.accs = {}
                    self.recs = {}
                    self.vps = None

                def s_and_consume(self):
                    # 4 PSUM tiles: (kk=jt-of-pair, ih=i-half)
                    for kk in range(2):
                        for ih in range(2):
                            self.s_tile(kk, ih)

                def s_tile(self, kk, ih):
                    # ACT: one [128 j, 1024 i] fill + exp consumer.
                    # DVE: ih indexes QUARTERS [128, 512] (8 tiles/unit).
                    jt = 2 * self.t + kk
                    if self.eng == 'A':
                        ps = s_slot()
                        for ic in range(4):
                            i0 = ih * 1024 + ic * 256
                            nc.tensor.matmul(
                                ps[:, ic * 256:(ic + 1) * 256],
                                lhsT=qh8[:, self.ht_k, :,
                                         jt * 128:(jt + 1) * 128],
                                rhs=qh8[:, self.ht_q, :, i0:i0 + 256],
                                start=True, stop=True, perf_mode=DR,
                            )
                        acc = smalls.tile([128, 1], F32, tag="acc",
                                          bufs=12, name="acc")
                        nc.scalar.activation(
                            self.e8[:, kk, ih * 1024:(ih + 1) * 1024],
                            ps[:], AF.Exp, scale=SCALE,
                            bias=expb_sb[:], accum_out=acc,
                        )
                        self.accs[(kk, ih)] = acc
                    else:
                        for q in range(2):
                            ps = sd_slot()
                            for ic in range(2):
                                i0 = ih * 1024 + q * 512 + ic * 256
                                nc.tensor.matmul(
                                    ps[:, ic * 256:(ic + 1) * 256],
                                    lhsT=qh8[:, self.ht_k, :,
                                             jt * 128:(jt + 1) * 128],
                                    rhs=qh8[:, self.ht_q, :, i0:i0 + 256],
                                    start=True, stop=True, perf_mode=DR,
                                )
                            i0 = ih * 1024 + q * 512
                            nc.vector.tensor_scalar(
                                self.e16[:, kk, i0:i0 + 512],
                                ps[:], A_SCH, B_SCH, ALU.mult, ALU.add)

                def sums_and_vp(self):
                    vps = vppool.tile([128, 2, 64], F8 if self.eng == 'A'
                                      else F16, tag="vp", name="vp")
                    self.vps = vps
                    for kk in range(2):
                        jt = 2 * self.t + kk
                        ssum = smalls.tile([128, 1], F32, tag="ssum", bufs=12,
                                           name="ssum")
                        if self.eng == 'A':
                            nc.gpsimd.tensor_add(
                                ssum[:], self.accs[(kk, 0)][:],
                                self.accs[(kk, 1)][:])
                        else:
                            nc.vector.tensor_scalar(
                                scr16[:], self.e16[:, kk, :].bitcast(F16),
                                1.0, None, ALU.mult, ALU.add,
                                accum_out=ssum[:])
                        rec = smalls.tile([128, 1], F32, tag="rec", bufs=12,
                                          name="rec")
                        nc.vector.reciprocal(rec[:], ssum[:])
                        lv = 2 * self.pair + self.h
                        nc.gpsimd.tensor_scalar(
                            vps[:, kk, :],
                            v16[:, jt, lv * 64:(lv + 1) * 64],
                            rec[:], VP, ALU.mult, ALU.mult,
                        )

                def pv(self, iq, ohs):
                    stop = (self.t == 7)
                    o_ps = ohs[self.h]
                    if self.eng == 'A':
                        for ic in range(2):
                            i0 = iq * 512 + ic * 256
                            nc.tensor.matmul(
                                o_ps[:, ic * 256:(ic + 1) * 256],
                                lhsT=self.vps[:],
                                rhs=self.e8[:, :, i0:i0 + 256],
                                start=False, stop=stop, perf_mode=DR,
                                skip_group_check=True,
                            )
                    else:
                        ef = self.e16[:].bitcast(F16)
                        i0 = iq * 512
                        for kk in range(2):
                            nc.tensor.matmul(
                                o_ps[:],
                                lhsT=self.vps[:, kk, :],
                                rhs=ef[:, kk, i0:i0 + 512],
                                start=False, stop=(stop and kk == 1),
                                skip_group_check=True,
                            )

            def oh_alloc():
                # fresh per-head O-QUARTER accumulators [64, 512] (partition
                # 0-63: DR matmuls cannot target a partition offset).
                # zero-matmuls set has_written so every PV is an accumulate.
                ohs = []
                for h in range(2):
                    o_ps = pp.tile([64, 512], F32, tag=f"oh{h}", bufs=1,
                                   name=f"o_ps{h}")
                    nc.tensor.matmul(
                        o_ps[:], lhsT=zrow_sb[:1, :64], rhs=zrow_sb[:1, :],
                        start=True, stop=False, skip_group_check=True,
                    )
                    ohs.append(o_ps)
                return ohs

            def o_evac(pair, iq, ohs):
                for h in range(2):
                    nc.vector.tensor_scalar(
                        o8f_tile(pair, h)[:, iq * 512:(iq + 1) * 512],
                        ohs[h][:], OEV, None, ALU.mult)

            def o_fold(pair, iq):
                sl = slice(iq * 512, (iq + 1) * 512)
                for kk in range(2):
                    nc.sync.dma_start(
                        o8d_tile(pair)[:, kk, sl], o8f_tile(pair, kk)[:, sl])

            # ---------------- emission ----------------
            # prologue: pair0 q/k projections + reshape
            for ft in (0, 1):
                for ih in range(2):
                    qk_proj(ft, ih)
            qk_reshape(0, 0)
            qk_reshape(0, 1)
            v_proj(0)
            v_proj(1)
            v_proj(2)
            v_proj(3)

            # aux work queue: emitted interleaved under pair0 attention.
            # v_proj(jt) must be emitted before sums of unit t=jt//2.
            aux = [lambda n=nt: v_proj(n) for nt in range(4, 16)]
            for ft in (2, 3):
                for ih in range(2):
                    aux.append(lambda f=ft, i=ih: qk_proj(f, i))
            aux.append(lambda: qk_reshape(1, 0))
            aux.append(lambda: qk_reshape(1, 1))

            def pop_aux(k):
                for _ in range(k):
                    if aux:
                        aux.pop(0)()

            units = {}
            order = [(t, h) for t in range(8) for h in (0, 1)]
            TILES = [(0, 0), (0, 1), (1, 0), (1, 1)]

            def mk_groups(pair):
                us = [Unit(pair, t, h) for (t, h) in order]
                for u in us:
                    units[(pair, u.t, u.h)] = u
                aq = [u for u in us if u.eng == 'A']
                dq = [u for u in us if u.eng == 'D']
                gs = []
                while aq or dq:
                    g = []
                    if aq:
                        g.append(aq.pop(0))
                    if dq:
                        g.append(dq.pop(0))
                    gs.append(g)
                return gs

            state = {}

            def pair_quarters(pair, nxt_drain):
                """Emit quarters 1-3 of `pair` as a work queue: each item
                runs on its own lazily-allocated oh set; nxt_drain lets the
                caller interleave these under the next pair's groups."""
                q = []
                for iq in range(1, 4):
                    def alloc(p=pair, i=iq):
                        state[(p, i)] = oh_alloc()
                    q.append(alloc)
                    for t, h in order:
                        q.append(lambda p=pair, i=iq, tt=t, hh=h:
                                 units[(p, tt, hh)].pv(i, state[(p, i)]))
                    def fin(p=pair, i=iq):
                        o_evac(p, i, state[(p, i)])
                        o_fold(p, i)
                        nxt_drain.extend(
                            lambda c=cot, pp_=p, ii=i: out_proj(pp_, c, ii)
                            for cot in range(4))
                    q.append(fin)
                return q

            # ---- pair 0: interleaved groups, proj/v aux between ----
            oh0 = oh_alloc()
            pend = None
            for g in mk_groups(0):
                for kk, ih in TILES:
                    for u in g:
                        u.s_tile(kk, ih)
                if pend:
                    for u in pend:
                        u.pv(0, oh0)
                for u in g:
                    u.sums_and_vp()
                pend = g
                pop_aux(3)
            for u in pend:
                u.pv(0, oh0)
            while aux:
                pop_aux(1)
            o_evac(0, 0, oh0)
            o_fold(0, 0)
            q_op = [lambda c=cot: out_proj(0, c, 0) for cot in range(4)]
            q_p0 = pair_quarters(0, q_op)

            # ---- pair 1 groups, draining pair0 quarters + out-proj ----
            q_p1a = []

            def get_oh1():
                if state.get((1, 0)) is None:
                    state[(1, 0)] = oh_alloc()
                return state[(1, 0)]

            def drain(n_budget=6):
                n = 0
                while q_p0 and n < n_budget:
                    q_p0.pop(0)()
                    n += 1
                # pair1 pv0s only after ALL pair0 quarters: the oh-tag
                # rotation means an early (1,0) alloc would wedge PE's
                # wait queue behind pair0's last evac
                if not q_p0:
                    while q_p1a and n < n_budget + 2:
                        q_p1a.pop(0)()
                        n += 1
                    if not q_p1a and q_op:
                        q_op.pop(0)()

            for g in mk_groups(1):
                for kk, ih in TILES:
                    for u in g:
                        u.s_tile(kk, ih)
                for u in g:
                    u.sums_and_vp()
                    q_p1a.append(lambda p=u: p.pv(0, get_oh1()))
                drain()
            while q_p0 or q_p1a:
                drain()
            o_evac(1, 0, state[(1, 0)])
            o_fold(1, 0)
            q_op.extend(lambda c=cot: out_proj(1, c, 0) for cot in range(4))

            # ---- pair 1 quarters 1-3 + remaining out-proj tail ----
            q_p1 = pair_quarters(1, q_op)
            while q_p1:
                q_p1.pop(0)()
                if q_op:
                    q_op.pop(0)()
            while q_op:
                q_op.pop(0)()

    nc.compile()
    return nc


def get_nc():
    global _NC
    if _NC is None:
        _NC = _build_nc()
    return _NC


def core_inputs(x, Wp, bp, core):
    """Host-side shard prep for one core: b = core//2, g = core%2."""
    b, g = divmod(core, 2)
    E4 = ml_dtypes.float8_e4m3

    def to8(a):
        return np.ascontiguousarray(np.asarray(a, np.float32).astype(E4))

    xb = x[b]  # [C, N]
    # x8[p, kt, kk, n] = x[kt*128 + kk*64 + p, n]
    x8 = np.transpose(xb.reshape(4, 2, 64, N), (2, 0, 1, 3))

    # qk feature order: ftile ft = 2*pair + (0=q,1=k); within: hA d0-63, hB
    qidx = np.zeros((4, 128), np.int64)
    for pair in range(2):
        for which in range(2):
            ft = 2 * pair + which
            for lh in range(2):
                h = 4 * g + 2 * pair + lh
                base = h * 192 + which * 64
                qidx[ft, lh * 64:(lh + 1) * 64] = np.arange(base, base + 64)
    Wqk = Wp[qidx.reshape(-1)]            # [512 feat, C]
    # wqk8[p, kt, kk, ft, j] = Wqk[ft*128 + j, kt*128 + kk*64 + p]
    wqk8 = np.transpose(
        Wqk.reshape(4, 128, 4, 2, 64), (4, 2, 3, 0, 1))
    bqk = bp[qidx.reshape(-1)].reshape(4, 128).T  # [128, 4]

    vidx = np.concatenate([np.arange((4 * g + lh) * 192 + 128,
                                     (4 * g + lh) * 192 + 192)
                           for lh in range(4)])
    Wv = Wp[vidx]                          # [256, C]
    # wv8[p, kt, kk, f] = Wv[f, kt*128 + kk*64 + p]
    wv8 = np.transpose(Wv.reshape(256, 4, 2, 64), (3, 1, 2, 0))

    return {
        "x8": to8(x8),
        "wqk8": to8(wqk8),
        "bqk": np.ascontiguousarray(bqk.astype(np.float32)),
        "wv8": to8(wv8),
        "bpv": bp[vidx].astype(np.float16).reshape(1, 256),
        "ones": np.ones((1, 128), np.float16),
    }


def wo_inputs(Wo, core):
    g = core % 2
    E4 = ml_dtypes.float8_e4m3
    # wo8[p, pair, kk, c] = Wo[c, 256*g + pair*128 + kk*64 + p]
    Wog = Wo[:, 256 * g:256 * (g + 1)]     # [C, 256]
    wo8 = np.transpose(Wog.reshape(C, 2, 2, 64), (3, 1, 2, 0))
    return np.ascontiguousarray(np.asarray(wo8, np.float32).astype(E4))


def kernel(x, Wp, bp, Wo, bo):
    global LAST_RESULT
    x = np.asarray(x, dtype=np.float32)
    Wp = np.asarray(Wp, dtype=np.float32)
    bp = np.asarray(bp, dtype=np.float32)
    Wo = np.asarray(Wo, dtype=np.float32)
    bo = np.asarray(bo, dtype=np.float32)

    in_maps = []
    for core in range(N_CORES):
        m = core_inputs(x, Wp, bp, core)
        m["wo8"] = wo_inputs(Wo, core)
        in_maps.append(m)

    nc = get_nc()
    res = run_bass_kernel_spmd(
        nc, in_maps, core_ids=list(range(N_CORES)),
        trace=bool(int(os.environ.get("KERNEL_TRACE", "0"))),
    )
    LAST_RESULT = res
    result = np.empty((B, C, N), dtype=np.float32)
    for b in range(B):
        r0, r1 = res.results[2 * b], res.results[2 * b + 1]
        result[b] = (
            (r0["out_a"].astype(np.float32) + r0["out_b"].astype(np.float32)
             + r1["out_a"].astype(np.float32) + r1["out_b"].astype(np.float32))
            / HOST_DIV
            + x[b] + bo[:, None]
        )
    return result


# revision 41
# speedup vs baseline: 1.2034x; 1.0095x over previous
"""Trainium2 Bass kernel for nn_AttentionBlock (B=4, C=512, N=2048, H=8, DK=64).

Computation (see module docstring of the reference):
  xt = x.T; qkv = xt @ Wp.T + bp; per head: S[j,i] = k_j . q_i / 8,
  P = softmax over i (query axis => per-j rows of S^T), O = P^T-weighted
  v-mix, out = (O @ Wo.T + bo + xt).T.

Sharding: 8 cores = (batch b = core//2) x (head-group g = core%2, 4 heads).
Each core emits two f16 partial resT [C, N] tensors (one per head pair);
host sums partials (x8 scale), adds bias + residual.

Engine strategy (cost-model driven):
  - All heavy matmuls are fp8e4 DoubleRow (0.5 cyc/row): QK/V projections
    (host supplies x/W pre-packed [64, 2, .] k-tiles), S (q/k re-packed to
    [32, 2, N] via SBUF->SBUF DMA), PV (contracts jt PAIRS: K=256 as
    [128, 2, .]), out-projection (o folded to [64, 2, N] via DMA).
  - exp work is split across THREE engines per (jtpair, head) unit:
      ACT: native Exp (scale=1/8, bias=-2ln2 so E<=61 fits e4m3),
           fp8 E tiles + free accum_out row sums -> DR PV.
      DVE/Pool: Schraudolph bit-trick exp: i16 = floor(A*S + B16) is the
           bit pattern of f16(~exp(S/8)); sums via a DVE tensor_scalar
           accum pass (4x mode); PV in fp16 for these tiles.
  - Normalization folds into v: vp = v * (VP/sumE); PV accumulates
    VP-scaled O in PSUM; o-evac rescales by 8/VP into fp8; host /8.
  - PSUM: 2 rotating [128,1024] S-slots (ACT + DVE consumers) + two
    per-head [64,1024] O-half accumulators (DR matmuls cannot write at a
    partition offset); PV runs in two i-half passes.
"""

import math
import os
import numpy as np
import ml_dtypes

import concourse.bass as bass
import concourse.tile as tile
from concourse import bacc, mybir
from concourse.bass_utils import run_bass_kernel_spmd

F32 = mybir.dt.float32
F16 = mybir.dt.float16
F8 = mybir.dt.float8e4
I16 = mybir.dt.int16
AF = mybir.ActivationFunctionType
ALU = mybir.AluOpType
DR = mybir.MatmulPerfMode.DoubleRow

B, C, N = 4, 512, 2048
H, DK = 8, 64
N_CORES = 8
SCALE = DK ** -0.5              # 0.125
EXPB = -2.0 * math.log(2.0)     # ACT exp bias: E' = exp(z)/4 (max ~61 < 240)
A_SCH = 1024.0 * 1.4426950408889634 * SCALE   # Schraudolph slope on raw S
B_SCH = 15294.0                 # tuned offset (incl trunc+centering)
VP = 512.0                      # vp pre-scale (fp8 precision for v/sumE)
OEV = 8.0 / VP                  # o-evac rescale: o8 = 8*O_true; host /8
HOST_DIV = 8.0

# per (pair, jtpair t, head h): engine for the exp unit.
# 'A' = ACT native exp (fp8 E, DR PV); 'D' = DVE Schraudolph (f16 E, fp16
# PV). GPSIMD cannot read PSUM (BIR verifier), so Pool only gets the
# SBUF-side work: sum passes over f16 E tiles, vp scaling, ssum adds.
ASSIGN = [
    [('A','A'),('A','D'),('D','A'),('A','D'),('D','A'),('A','D'),('D','D'),('A','A')],
    [('A','A'),('A','D'),('D','A'),('A','D'),('D','A'),('A','D'),('D','A'),('A','A')],
]

LAST_RESULT = None
_NC = None


def _build_nc():
    nc = bacc.Bacc("TRN2", target_bir_lowering=False, debug=False,
                   num_devices=N_CORES)

    x8 = nc.dram_tensor("x8", [64, 4, 2, N], F8, kind="ExternalInput").ap()
    wqk8 = nc.dram_tensor("wqk8", [64, 4, 2, 4, 128], F8, kind="ExternalInput").ap()
    bqk = nc.dram_tensor("bqk", [128, 4], F32, kind="ExternalInput").ap()
    wv8 = nc.dram_tensor("wv8", [64, 4, 2, 256], F8, kind="ExternalInput").ap()
    bpv = nc.dram_tensor("bpv", [1, 256], F16, kind="ExternalInput").ap()
    wo8 = nc.dram_tensor("wo8", [64, 2, 2, C], F8, kind="ExternalInput").ap()
    ones = nc.dram_tensor("ones", [1, 128], F16, kind="ExternalInput").ap()
    out_a = nc.dram_tensor("out_a", [C, N], F16, kind="ExternalOutput").ap()
    out_b = nc.dram_tensor("out_b", [C, N], F16, kind="ExternalOutput").ap()

    with tile.TileContext(nc) as tc:
        with (
            tc.tile_pool(name="consts", bufs=1) as consts,
            tc.tile_pool(name="qkpool", bufs=1) as qkpool,
            tc.tile_pool(name="vpool", bufs=1) as vpool,
            tc.tile_pool(name="e8pool", bufs=11) as e8pool,
            tc.tile_pool(name="e16pool", bufs=7) as e16pool,
            tc.tile_pool(name="vppool", bufs=18) as vppool,
            tc.tile_pool(name="opool", bufs=2) as opool,
            tc.tile_pool(name="outpool", bufs=2) as outpool,
            tc.tile_pool(name="smalls", bufs=40) as smalls,
            tc.tile_pool(name="psum", bufs=1, space="PSUM") as pp,
        ):
            ones_sb = consts.tile([1, 128], F16)
            nc.sync.dma_start(ones_sb[:], ones[:])
            bqk_sb = consts.tile([128, 4], F32)
            nc.sync.dma_start(bqk_sb[:], bqk[:])
            bpv_sb = consts.tile([1, 256], F16)
            nc.sync.dma_start(bpv_sb[:], bpv[:])
            wqk_sb = consts.tile([64, 4, 2, 4, 128], F8)
            nc.sync.dma_start(wqk_sb[:], wqk8[:])
            x_kt = []
            for kt in range(4):
                t_ = consts.tile([64, 2, N], F8, name=f"x_kt{kt}")
                nc.sync.dma_start(t_[:], x8[:, kt])
                x_kt.append(t_)
            wv_sb = consts.tile([64, 4, 2, 256], F8)
            nc.sync.dma_start(wv_sb[:], wv8[:])
            wo_sb = consts.tile([64, 2, 2, C], F8)
            nc.sync.dma_start(wo_sb[:], wo8[:])

            # warm the ACT exp table while DMAs run
            warm = smalls.tile([1, 128], F16, tag="warm", bufs=1, name="warm")
            nc.scalar.activation(warm[:], ones_sb[:], AF.Exp)
            expb_sb = consts.tile([128, 1], F32)
            nc.vector.memset(expb_sb[:], EXPB)
            zrow_sb = consts.tile([1, 512], F16)
            nc.vector.memset(zrow_sb[:], 0.0)

            # persistent SBUF tensors
            # qk8: fp8 evac of the QK projection [128 feat, N], rotated
            qk8_of = {}

            def qk8_tile(ft):
                if ft not in qk8_of:
                    qk8_of[ft] = qkpool.tile([128, N], F8, tag="qk8e",
                                             bufs=2, name="qk8e")
                return qk8_of[ft]
            # qh8: S-DR layout [32, (head,qk) 8, kk 2, N]
            qh8 = qkpool.tile([32, 8, 2, N], F8, name="qh8")
            v16 = vpool.tile([128, 16, 256], F16, name="v16")
            # o8f: per-pair, per-head fp8 o evac [64 d, N] (partition 0-63)
            o8f_of = {}

            def o8f_tile(p, h):
                if (p, h) not in o8f_of:
                    o8f_of[(p, h)] = opool.tile([64, N], F8, tag=f"o8f{h}",
                                                bufs=1, name="o8f")
                return o8f_of[(p, h)]
            # o8d: DR-folded [64, 2, N]
            o8d_of = {}

            def o8d_tile(p):
                if p not in o8d_of:
                    o8d_of[p] = opool.tile([64, 2, N], F8, tag="o8d", bufs=1,
                                           name="o8d")
                return o8d_of[p]
            scr16 = qkpool.tile([128, 2048], F16, name="scr16")

            def s_slot():
                # ACT-dedicated slots (also used by proj/outproj fills)
                return pp.tile([128, 1024], F32, tag="s", bufs=2, name="s_ps")

            def sd_slot():
                # DVE-dedicated half-size slots: decouples the DVE consumer
                # chain from ACT's so neither stalls the other
                return pp.tile([128, 512], F32, tag="sd", bufs=2,
                               name="sd_ps")

            def qk_proj(ft, ih):
                # one i-half of ftile ft -> PSUM -> fp8 evac into qk8[:, ft]
                ps = s_slot()
                for ic in range(4):
                    nch = ih * 1024 + ic * 256
                    for kt in range(4):
                        nc.tensor.matmul(
                            ps[:, ic * 256:(ic + 1) * 256],
                            lhsT=wqk_sb[:, kt, :, ft, :],
                            rhs=x_kt[kt][:, :, nch:nch + 256],
                            start=(kt == 0), stop=(kt == 3), perf_mode=DR,
                        )
                nc.scalar.activation(
                    qk8_tile(ft)[:, ih * 1024:(ih + 1) * 1024],
                    ps[:], AF.Identity, bias=bqk_sb[:, ft:ft + 1])

            def qk_reshape(pair, which, ih):
                # fold [128, N] ftile into S-DR layout [32, 2, N] per head;
                # per i-half so the first S can start early
                ft = 2 * pair + which
                src = qk8_tile(ft)
                sl = slice(ih * 1024, (ih + 1) * 1024)
                for lh in range(2):
                    ht = (2 * pair + lh) * 2 + which
                    for kk in range(2):
                        base = lh * 64 + kk * 32
                        nc.sync.dma_start(
                            qh8[:, ht, kk, sl], src[base:base + 32, sl])

            def v_proj(nt):
                ps = s_slot()
                for kt in range(4):
                    nc.tensor.matmul(
                        ps[:, :256],
                        lhsT=x_kt[kt][:, :, nt * 128:(nt + 1) * 128],
                        rhs=wv_sb[:, kt], start=(kt == 0), stop=False,
                        perf_mode=DR,
                    )
                nc.tensor.matmul(
                    ps[:, :256], lhsT=ones_sb[:1, :], rhs=bpv_sb[:1, :],
                    start=False, stop=True,
                )
                if nt % 2 == 0:
                    nc.scalar.copy(v16[:, nt, :], ps[:, :256])
                else:
                    nc.vector.tensor_copy(v16[:, nt, :], ps[:, :256])

            op_count = [0]

            def out_proj(pair, cot, iq):
                # [128 c, 512 i] DR out-proj chunk + f16 evac + DMA
                dst = out_a if pair == 0 else out_b
                ps = s_slot()
                for ic in range(2):
                    i0 = iq * 512 + ic * 256
                    nc.tensor.matmul(
                        ps[:, ic * 256:(ic + 1) * 256],
                        lhsT=wo_sb[:, pair, :, cot * 128:(cot + 1) * 128],
                        rhs=o8d_tile(pair)[:, :, i0:i0 + 256],
                        start=True, stop=True, perf_mode=DR,
                    )
                ot = outpool.tile([128, 512], F16, tag="out", bufs=4,
                                  name="out_t")
                eng = [nc.scalar.copy,
                       nc.vector.tensor_copy][op_count[0] % 2]
                op_count[0] += 1
                eng(ot[:], ps[:, :512])
                nc.sync.dma_start(
                    dst[cot * 128:(cot + 1) * 128,
                        iq * 512:(iq + 1) * 512], ot[:])

            class Unit:
                """One (pair, jtpair, head) exp unit."""

                def __init__(self, pair, t, h):
                    self.pair, self.t, self.h = pair, t, h
                    self.eng = ASSIGN[pair][t][h]
                    self.ht_q = (2 * pair + h) * 2
                    self.ht_k = self.ht_q + 1
                    if self.eng == 'A':
                        self.e8 = e8pool.tile([128, 2, N], F8, tag="e8",
                                              name="e8")
                    else:
                        self.e16 = e16pool.tile([128, 2, N], I16, tag="e16",
                                                name="e16")
                    self.accs = {}
                    self.recs = {}
                    self.vps = None

                def s_and_consume(self):
                    # 4 PSUM tiles: (kk=jt-of-pair, ih=i-half)
                    for kk in range(2):
                        for ih in range(2):
                            self.s_tile(kk, ih)

                def s_tile(self, kk, ih):
                    # ACT: one [128 j, 1024 i] fill + exp consumer.
                    # DVE: ih indexes QUARTERS [128, 512] (8 tiles/unit).
                    jt = 2 * self.t + kk
                    if self.eng == 'A':
                        ps = s_slot()
                        for ic in range(4):
                            i0 = ih * 1024 + ic * 256
                            nc.tensor.matmul(
                                ps[:, ic * 256:(ic + 1) * 256],
                                lhsT=qh8[:, self.ht_k, :,
                                         jt * 128:(jt + 1) * 128],
                                rhs=qh8[:, self.ht_q, :, i0:i0 + 256],
                                start=True, stop=True, perf_mode=DR,
                            )
                        acc = smalls.tile([128, 1], F32, tag="acc",
                                          bufs=12, name="acc")
                        nc.scalar.activation(
                            self.e8[:, kk, ih * 1024:(ih + 1) * 1024],
                            ps[:], AF.Exp, scale=SCALE,
                            bias=expb_sb[:], accum_out=acc,
                        )
                        self.accs[(kk, ih)] = acc
                    else:
                        for q in range(2):
                            ps = sd_slot()
                            for ic in range(2):
                                i0 = ih * 1024 + q * 512 + ic * 256
                                nc.tensor.matmul(
                                    ps[:, ic * 256:(ic + 1) * 256],
                                    lhsT=qh8[:, self.ht_k, :,
                                             jt * 128:(jt + 1) * 128],
                                    rhs=qh8[:, self.ht_q, :, i0:i0 + 256],
                                    start=True, stop=True, perf_mode=DR,
                                )
                            i0 = ih * 1024 + q * 512
                            nc.vector.tensor_scalar(
                                self.e16[:, kk, i0:i0 + 512],
                                ps[:], A_SCH, B_SCH, ALU.mult, ALU.add)

                def sums_and_vp(self):
                    vps = vppool.tile([128, 2, 64], F8 if self.eng == 'A'
                                      else F16, tag="vp", name="vp")
                    self.vps = vps
                    for kk in range(2):
                        jt = 2 * self.t + kk
                        ssum = smalls.tile([128, 1], F32, tag="ssum", bufs=12,
                                           name="ssum")
                        if self.eng == 'A':
                            nc.gpsimd.tensor_add(
                                ssum[:], self.accs[(kk, 0)][:],
                                self.accs[(kk, 1)][:])
                        else:
                            nc.vector.tensor_scalar(
                                scr16[:], self.e16[:, kk, :].bitcast(F16),
                                1.0, None, ALU.mult, ALU.add,
                                accum_out=ssum[:])
                        rec = smalls.tile([128, 1], F32, tag="rec", bufs=12,
                                          name="rec")
                        nc.vector.reciprocal(rec[:], ssum[:])
                        lv = 2 * self.pair + self.h
                        nc.gpsimd.tensor_scalar(
                            vps[:, kk, :],
                            v16[:, jt, lv * 64:(lv + 1) * 64],
                            rec[:], VP, ALU.mult, ALU.mult,
                        )

                def pv(self, iq, ohs):
                    stop = (self.t == 7)
                    o_ps = ohs[self.h]
                    if self.eng == 'A':
                        for ic in range(2):
                            i0 = iq * 512 + ic * 256
                            nc.tensor.matmul(
                                o_ps[:, ic * 256:(ic + 1) * 256],
                                lhsT=self.vps[:],
                                rhs=self.e8[:, :, i0:i0 + 256],
                                start=False, stop=stop, perf_mode=DR,
                                skip_group_check=True,
                            )
                    else:
                        ef = self.e16[:].bitcast(F16)
                        i0 = iq * 512
                        for kk in range(2):
                            nc.tensor.matmul(
                                o_ps[:],
                                lhsT=self.vps[:, kk, :],
                                rhs=ef[:, kk, i0:i0 + 512],
                                start=False, stop=(stop and kk == 1),
                                skip_group_check=True,
                            )

            def oh_alloc():
                # fresh per-head O-QUARTER accumulators [64, 512] (partition
                # 0-63: DR matmuls cannot target a partition offset).
                # zero-matmuls set has_written so every PV is an accumulate.
                ohs = []
                for h in range(2):
                    o_ps = pp.tile([64, 512], F32, tag=f"oh{h}", bufs=1,
                                   name=f"o_ps{h}")
                    nc.tensor.matmul(
                        o_ps[:], lhsT=zrow_sb[:1, :64], rhs=zrow_sb[:1, :],
                        start=True, stop=False, skip_group_check=True,
                    )
                    ohs.append(o_ps)
                return ohs

            def o_evac(pair, iq, ohs):
                for h in range(2):
                    nc.vector.tensor_scalar(
                        o8f_tile(pair, h)[:, iq * 512:(iq + 1) * 512],
                        ohs[h][:], OEV, None, ALU.mult)

            def o_fold(pair, iq):
                sl = slice(iq * 512, (iq + 1) * 512)
                for kk in range(2):
                    nc.sync.dma_start(
                        o8d_tile(pair)[:, kk, sl], o8f_tile(pair, kk)[:, sl])

            # ---------------- emission ----------------
            # aux work queue: emitted interleaved under pair0 attention.
            # v_proj(jt) must be emitted before sums of unit t=jt//2;
            # ih=1 proj/reshape halves must precede the (kk, 1) tiles.
            aux = [lambda n=nt: v_proj(n) for nt in range(4, 16)]
            for ft in (2, 3):
                for ih in range(2):
                    aux.append(lambda f=ft, i=ih: qk_proj(f, i))
            for ih in range(2):
                aux.append(lambda i=ih: qk_reshape(1, 0, i))
                aux.append(lambda i=ih: qk_reshape(1, 1, i))

            def pop_aux(k):
                for _ in range(k):
                    if aux:
                        aux.pop(0)()

            units = {}
            order = [(t, h) for t in range(8) for h in (0, 1)]
            TILES = [(0, 0), (1, 0), (0, 1), (1, 1)]

            def mk_groups(pair):
                us = [Unit(pair, t, h) for (t, h) in order]
                for u in us:
                    units[(pair, u.t, u.h)] = u
                aq = [u for u in us if u.eng == 'A']
                dq = [u for u in us if u.eng == 'D']
                gs = []
                while aq or dq:
                    g = []
                    if aq:
                        g.append(aq.pop(0))
                    if dq:
                        g.append(dq.pop(0))
                    gs.append(g)
                return gs

            state = {}

            def pair_quarters(pair, nxt_drain):
                """Emit quarters 1-3 of `pair` as a work queue: each item
                runs on its own lazily-allocated oh set; nxt_drain lets the
                caller interleave these under the next pair's groups."""
                q = []
                for iq in range(1, 4):
                    def alloc(p=pair, i=iq):
                        state[(p, i)] = oh_alloc()
                    q.append(alloc)
                    for h in (0, 1):
                        for t, h2 in order:
                            if h2 != h:
                                continue
                            q.append(lambda p=pair, i=iq, tt=t, hh=h2:
                                     units[(p, tt, hh)].pv(i, state[(p, i)]))
                        def evh(p=pair, i=iq, hh=h):
                            nc.vector.tensor_scalar(
                                o8f_tile(p, hh)[:, i * 512:(i + 1) * 512],
                                state[(p, i)][hh][:], OEV, None, ALU.mult)
                        q.append(evh)
                    def fin(p=pair, i=iq):
                        o_fold(p, i)
                        nxt_drain.extend(
                            lambda c=cot, pp_=p, ii=i: out_proj(pp_, c, ii)
                            for cot in range(4))
                    q.append(fin)
                return q

            # ---- pair 0: interleaved groups, proj/v aux between ----
            qk_proj(0, 0)
            qk_reshape(0, 0, 0)
            qk_proj(1, 0)
            qk_reshape(0, 1, 0)
            qk_proj(0, 1)
            qk_reshape(0, 0, 1)
            qk_proj(1, 1)
            qk_reshape(0, 1, 1)
            v_proj(0)
            v_proj(1)
            v_proj(2)
            v_proj(3)
            oh0 = oh_alloc()
            pend = None
            for g in mk_groups(0):
                for kk, ih in TILES:
                    for u in g:
                        u.s_tile(kk, ih)
                if pend:
                    for u in pend:
                        u.pv(0, oh0)
                for u in g:
                    u.sums_and_vp()
                pend = g
                pop_aux(3)
            for u in pend:
                u.pv(0, oh0)
            while aux:
                pop_aux(1)
            o_evac(0, 0, oh0)
            o_fold(0, 0)
            q_op = [lambda c=cot: out_proj(0, c, 0) for cot in range(4)]
            q_p0 = pair_quarters(0, q_op)

            # ---- pair 1 groups, draining pair0 quarters + out-proj ----
            q_p1a = []

            def get_oh1():
                if state.get((1, 0)) is None:
                    state[(1, 0)] = oh_alloc()
                return state[(1, 0)]

            def drain(n_budget=6):
                n = 0
                while q_p0 and n < n_budget:
                    q_p0.pop(0)()
                    n += 1
                # pair1 pv0s only after ALL pair0 quarters: the oh-tag
                # rotation means an early (1,0) alloc would wedge PE's
                # wait queue behind pair0's last evac
                if not q_p0:
                    while q_p1a and n < n_budget + 2:
                        q_p1a.pop(0)()
                        n += 1
                    if not q_p1a and q_op:
                        q_op.pop(0)()

            for g in mk_groups(1):
                for kk, ih in TILES:
                    for u in g:
                        u.s_tile(kk, ih)
                for u in g:
                    u.sums_and_vp()
                    q_p1a.append(lambda p=u: p.pv(0, get_oh1()))
                drain()
            while q_p0 or q_p1a:
                drain()
            o_evac(1, 0, state[(1, 0)])
            o_fold(1, 0)
            q_op.extend(lambda c=cot: out_proj(1, c, 0) for cot in range(4))

            # ---- pair 1 quarters 1-3 + remaining out-proj tail ----
            q_p1 = pair_quarters(1, q_op)
            while q_p1:
                q_p1.pop(0)()
                if q_op:
                    q_op.pop(0)()
            while q_op:
                q_op.pop(0)()

    nc.compile()
    return nc


def get_nc():
    global _NC
    if _NC is None:
        _NC = _build_nc()
    return _NC


def core_inputs(x, Wp, bp, core):
    """Host-side shard prep for one core: b = core//2, g = core%2."""
    b, g = divmod(core, 2)
    E4 = ml_dtypes.float8_e4m3

    def to8(a):
        return np.ascontiguousarray(np.asarray(a, np.float32).astype(E4))

    xb = x[b]  # [C, N]
    # x8[p, kt, kk, n] = x[kt*128 + kk*64 + p, n]
    x8 = np.transpose(xb.reshape(4, 2, 64, N), (2, 0, 1, 3))

    # qk feature order: ftile ft = 2*pair + (0=q,1=k); within: hA d0-63, hB
    qidx = np.zeros((4, 128), np.int64)
    for pair in range(2):
        for which in range(2):
            ft = 2 * pair + which
            for lh in range(2):
                h = 4 * g + 2 * pair + lh
                base = h * 192 + which * 64
                qidx[ft, lh * 64:(lh + 1) * 64] = np.arange(base, base + 64)
    Wqk = Wp[qidx.reshape(-1)]            # [512 feat, C]
    # wqk8[p, kt, kk, ft, j] = Wqk[ft*128 + j, kt*128 + kk*64 + p]
    wqk8 = np.transpose(
        Wqk.reshape(4, 128, 4, 2, 64), (4, 2, 3, 0, 1))
    bqk = bp[qidx.reshape(-1)].reshape(4, 128).T  # [128, 4]

    vidx = np.concatenate([np.arange((4 * g + lh) * 192 + 128,
                                     (4 * g + lh) * 192 + 192)
                           for lh in range(4)])
    Wv = Wp[vidx]                          # [256, C]
    # wv8[p, kt, kk, f] = Wv[f, kt*128 + kk*64 + p]
    wv8 = np.transpose(Wv.reshape(256, 4, 2, 64), (3, 1, 2, 0))

    return {
        "x8": to8(x8),
        "wqk8": to8(wqk8),
        "bqk": np.ascontiguousarray(bqk.astype(np.float32)),
        "wv8": to8(wv8),
        "bpv": bp[vidx].astype(np.float16).reshape(1, 256),
        "ones": np.ones((1, 128), np.float16),
    }


def wo_inputs(Wo, core):
    g = core % 2
    E4 = ml_dtypes.float8_e4m3
    # wo8[p, pair, kk, c] = Wo[c, 256*g + pair*128 + kk*64 + p]
    Wog = Wo[:, 256 * g:256 * (g + 1)]     # [C, 256]
    wo8 = np.transpose(Wog.reshape(C, 2, 2, 64), (3, 1, 2, 0))
    return np.ascontiguousarray(np.asarray(wo8, np.float32).astype(E4))


def kernel(x, Wp, bp, Wo, bo):
    global LAST_RESULT
    x = np.asarray(x, dtype=np.float32)
    Wp = np.asarray(Wp, dtype=np.float32)
    bp = np.asarray(bp, dtype=np.float32)
    Wo = np.asarray(Wo, dtype=np.float32)
    bo = np.asarray(bo, dtype=np.float32)

    in_maps = []
    for core in range(N_CORES):
        m = core_inputs(x, Wp, bp, core)
        m["wo8"] = wo_inputs(Wo, core)
        in_maps.append(m)

    nc = get_nc()
    res = run_bass_kernel_spmd(
        nc, in_maps, core_ids=list(range(N_CORES)),
        trace=bool(int(os.environ.get("KERNEL_TRACE", "0"))),
    )
    LAST_RESULT = res
    result = np.empty((B, C, N), dtype=np.float32)
    for b in range(B):
        r0, r1 = res.results[2 * b], res.results[2 * b + 1]
        result[b] = (
            (r0["out_a"].astype(np.float32) + r0["out_b"].astype(np.float32)
             + r1["out_a"].astype(np.float32) + r1["out_b"].astype(np.float32))
            / HOST_DIV
            + x[b] + bo[:, None]
        )
    return result
